# revision 1
# baseline (speedup 1.0000x reference)
"""DGMNet (dense MLP, 4 DGM layers) Trainium2 kernel.

Strategy: data-parallel over the batch dim (65536 rows -> 8 cores x 8192).
Inside each core, activations live feature-major in SBUF ([128 features x
batch-tile] tiles), so every gate matmul is out[M=feat,N=batch] =
W.T-slice @ S with PE accumulation over the 1024-feature contraction, the
x-side projections (K=16) are folded into the same PSUM accumulation
group (issued 4-wide via row tiling so they cost ~1/4), and biases ride
the ACT tanh for free. The scalar output row of tile t is computed during
tile t+1's S1/G phase so the PE never waits on the layer-3 tail.

Host-side preprocessing (numpy): transpose x and the weight matrices so
the kernel never transposes on-device, replicate the 16-row U block at
partitions 0/32/64/96 for row-tiled matmuls, and fold the U/W bias pairs.
"""

import sys

sys.path.insert(0, "/opt/trn_rl_repo")

import numpy as np

B_FULL = 65536
KI = 16
H = 1024
NCORES = 8
BC = B_FULL // NCORES  # per-core batch (8192)
NB = 512               # batch tile (one PSUM bank of fp32)
NM = H // 128          # feature tiles (8)
N_LAYERS = 4

# float32r streams fp32 data through the PE at 1 column/cycle (4x the fp32
# rate) with reduced internal mantissa; float32 is the exact-but-slow mode.
MM_DT = "float32r"

_BUILD_CACHE = {}


def _build(bc, nb, mm_dt, repeat=1, pack_k16=True, out_slot="s1", s1_act=True, share_wgs=True):
    """Build + compile the single-core Bass program. Returns nc.

    repeat > 1 re-runs the whole computation (for slope-based timing of the
    device execution under the large axon dispatch overhead)."""
    import concourse.bacc as bacc
    import concourse.mybir as mybir
    import concourse.tile as tile

    f32 = mybir.dt.float32
    mdt = getattr(mybir.dt, mm_dt)
    Tanh = mybir.ActivationFunctionType.Tanh
    Ident = mybir.ActivationFunctionType.Identity
    mult = mybir.AluOpType.mult
    add = mybir.AluOpType.add

    nt = bc // nb

    nc = bacc.Bacc("TRN2", target_bir_lowering=False, debug=False,
                   num_devices=NCORES)

    xT_d = nc.dram_tensor("xT", [KI, bc], mdt, kind="ExternalInput").ap()
    wz_d = nc.dram_tensor("WzT", [H, H], mdt, kind="ExternalInput").ap()
    wg_d = nc.dram_tensor("WgT", [H, H], mdt, kind="ExternalInput").ap()
    u_d = nc.dram_tensor("U", [128, 5 * H], mdt, kind="ExternalInput").ap()
    bias_d = nc.dram_tensor("BIAS", [128, 48], f32, kind="ExternalInput").ap()
    ow_d = nc.dram_tensor("OW", [128, NM], mdt, kind="ExternalInput").ap()
    y_d = nc.dram_tensor("Y", [1, bc], f32, kind="ExternalOutput").ap()

    with tile.TileContext(nc) as tc:
        with (
            tc.tile_pool(name="const", bufs=1) as cpool,
            tc.tile_pool(name="xt", bufs=3) as xt_pool,
            tc.tile_pool(name="s", bufs=2) as s_pool,
            tc.tile_pool(name="act", bufs=1) as act_pool,
            tc.tile_pool(name="ov", bufs=2) as ov_pool,
            tc.tile_pool(name="psum", bufs=7, space="PSUM") as ps_pool,
            tc.tile_pool(name="pso", bufs=1, space="PSUM") as pso_pool,
        ):
            # ---- resident constants (small ones first; G needs Wg before
            # Z needs Wz, so load Wg ahead of Wz) -------------------------
            u_sb = cpool.tile([128, 5 * H], mdt)
            nc.gpsimd.dma_start(u_sb[:], u_d[:])
            bias_sb = cpool.tile([128, 48], f32)
            nc.gpsimd.dma_start(bias_sb[:], bias_d[:])
            ow_sb = cpool.tile([128, NM], mdt)
            nc.gpsimd.dma_start(ow_sb[:], ow_d[:])
            wz_sb = cpool.tile([128, NM * H], mdt)
            wg_sb = cpool.tile([128, NM * H], mdt)
            for k in range(NM):
                nc.gpsimd.dma_start(wg_sb[:, k * H:(k + 1) * H],
                                    wg_d[k * 128:(k + 1) * 128, :])
            for k in range(NM):
                nc.gpsimd.dma_start(wz_sb[:, k * H:(k + 1) * H],
                                    wz_d[k * 128:(k + 1) * 128, :])

            def w_ap(w_sb, k, m):
                return w_sb[:, k * H + m * 128:k * H + (m + 1) * 128]

            def u_ap(g, m, c):
                return u_sb[32 * c:32 * c + KI,
                            g * H + m * 128:g * H + (m + 1) * 128]

            def b_ap(g, m):
                return bias_sb[:, g * NM + m:g * NM + m + 1]

            def x_starts(gate, xt, nb, nametag, single):
                """Row-tiled (4-concurrent) K=16 start matmuls for m-quads.
                PSUM tiles are allocated per quad so the second quad never
                waits on pool slots freed by the first quad's evacuation."""
                import concourse.mybir as _mb
                pss = [None] * NM
                for mq in (0, 4):
                    for c in range(4):
                        m = mq + c
                        pss[m] = ps_pool.tile([128, nb], _mb.dt.float32,
                                              tag="ps",
                                              name=f"{nametag}_{m}")
                    for c in range(4):
                        m = mq + c
                        cc = c if pack_k16 else 0
                        nc.tensor.matmul(
                            pss[m][:], u_ap(gate, m, cc),
                            xt[32 * cc:32 * cc + KI, :],
                            start=True, stop=single,
                            tile_position=(32 * cc, 0))
                return pss

            # ---- per batch tile -----------------------------------------
            pend = None  # deferred output row of the previous batch tile

            def emit_out(pend):
                h_prev, tp, up = pend
                po = pso_pool.tile([1, nb], f32, tag="po", name=f"po_{up}")
                for k in range(NM):
                    nc.tensor.matmul(po[:], ow_sb[:, k:k + 1], h_prev[k][:],
                                     start=(k == 0), stop=(k == NM - 1))
                orow = ov_pool.tile([1, nb], f32, tag="orow", name=f"orow_{up}")
                nc.vector.tensor_scalar_add(orow[:], po[:],
                                            bias_sb[0:1, 40:41])
                nc.gpsimd.dma_start(y_d[0:1, tp * nb:(tp + 1) * nb], orow[:])

            for rep in range(repeat):
                for t in range(nt):
                    t_u = rep * nt + t  # unique suffix for tile names
                    xt = xt_pool.tile([128, nb], mdt, tag="xt",
                                      name=f"xt_{t_u}")
                    for c in range(4):
                        nc.gpsimd.dma_start(xt[32 * c:32 * c + KI, :],
                                            xT_d[:, t * nb:(t + 1) * nb])

                    # S1 = x @ Sw.T + Sw_b (no tanh; evacuate via ACT
                    # Identity so the DVE stays free for the combines)
                    s_cur = [s_pool.tile([128, nb], mdt, tag=f"s{k}",
                                         name=f"s_{t_u}_0_{k}")
                             for k in range(NM)]
                    ps1 = x_starts(0, xt, nb, f"ps_s1_{t_u}", single=True)
                    for m in range(NM):
                        if s1_act:
                            nc.scalar.activation(s_cur[m][:], ps1[m][:],
                                                 Ident, bias=b_ap(0, m))
                        else:
                            nc.vector.tensor_scalar_add(s_cur[m][:],
                                                        ps1[m][:], b_ap(0, m))

                    # previous tile's output row fills the S1->G dep gap
                    if pend is not None and out_slot == "s1":
                        emit_out(pend)
                        pend = None

                    # wgS1 = Wg @ S1, shared by G and the layer-0 R
                    # gate (the reference reuses it); parked in the h slots,
                    # which are free until layer 0's H.
                    if share_wgs:
                        wgs = [act_pool.tile([128, nb], mdt, tag=f"h{m}",
                                             name=f"wgs_{t_u}_{m}")
                               for m in range(NM)]
                        for m in range(NM):
                            ps = ps_pool.tile([128, nb], f32, tag="ps",
                                              name=f"ps_wgs_{t_u}_{m}")
                            for k in range(NM):
                                nc.tensor.matmul(ps[:], w_ap(wg_sb, k, m),
                                                 s_cur[k][:],
                                                 start=(k == 0),
                                                 stop=(k == NM - 1))
                            nc.scalar.activation(wgs[m][:], ps[:], Ident)
    

                    if pend is not None and out_slot == "g":
                        emit_out(pend)
                        pend = None

                    for i in range(N_LAYERS):
                        # R = tanh(br + Ur x + Wg S); layer 0 reuses wgS1
                        r_t = [act_pool.tile([128, nb], mdt, tag=f"r{m}",
                                             name=f"r_{t_u}_{i}_{m}")
                               for m in range(NM)]
                        if i == 0 and not share_wgs:
                            # unshared fallback: full Wg@S1 groups for G and R
                            g_t = [act_pool.tile([128, nb], f32, tag=f"g{m}",
                                                 name=f"g_{t_u}_{m}")
                                   for m in range(NM)]
                            for gate, dest, bgi in ((2, g_t, 2), (3, r_t, 3)):
                                for mq in (0, 4):
                                    pss = {mq + c: ps_pool.tile(
                                        [128, nb], f32, tag="ps",
                                        name=f"ps_u{gate}_{t_u}_{mq + c}")
                                        for c in range(4)}
                                    for c in range(4):
                                        m = mq + c
                                        cc = c if pack_k16 else 0
                                        nc.tensor.matmul(
                                            pss[m][:], u_ap(gate, m, cc),
                                            xt[32 * cc:32 * cc + KI, :],
                                            start=True, stop=False,
                                            tile_position=(32 * cc, 0))
                                    for c in range(4):
                                        m = mq + c
                                        for k in range(NM):
                                            nc.tensor.matmul(
                                                pss[m][:], w_ap(wg_sb, k, m),
                                                s_cur[k][:],
                                                start=False,
                                                stop=(k == NM - 1))
                                        nc.scalar.activation(
                                            dest[m][:], pss[m][:], Tanh,
                                            bias=b_ap(bgi, m))
                        elif i == 0:
                            ps_r = x_starts(3, xt, nb, f"ps_r_{t_u}_0",
                                            single=True)
                            for m in range(NM):
                                nc.vector.tensor_add(r_t[m][:], ps_r[m][:],
                                                     wgs[m][:])
                                nc.scalar.activation(r_t[m][:], r_t[m][:],
                                                     Tanh, bias=b_ap(3, m))
                            # G = tanh(bg + Ug x + wgS1); the (1-G) transform
                            # is deferred past the H matmuls to keep the DVE
                            # off the H-gate critical path
                            g_t = [act_pool.tile([128, nb], f32, tag=f"g{m}",
                                                 name=f"g_{t_u}_{m}")
                                   for m in range(NM)]
                            ps_g = x_starts(2, xt, nb, f"ps_g_{t_u}",
                                            single=True)
                            for m in range(NM):
                                nc.vector.tensor_add(g_t[m][:], ps_g[m][:],
                                                     wgs[m][:])
                                nc.scalar.activation(g_t[m][:], g_t[m][:],
                                                     Tanh, bias=b_ap(2, m))
                        else:
                            for mq in (0, 4):
                                pss = {mq + c: ps_pool.tile(
                                    [128, nb], f32, tag="ps",
                                    name=f"ps_r_{t_u}_{i}_{mq + c}")
                                    for c in range(4)}
                                for c in range(4):
                                    m = mq + c
                                    cc = c if pack_k16 else 0
                                    nc.tensor.matmul(
                                        pss[m][:], u_ap(3, m, cc),
                                        xt[32 * cc:32 * cc + KI, :],
                                        start=True, stop=False,
                                        tile_position=(32 * cc, 0))
                                for c in range(4):
                                    m = mq + c
                                    for k in range(NM):
                                        nc.tensor.matmul(
                                            pss[m][:], w_ap(wg_sb, k, m),
                                            s_cur[k][:],
                                            start=False, stop=(k == NM - 1))
                                    nc.scalar.activation(r_t[m][:], pss[m][:],
                                                         Tanh, bias=b_ap(3, m))

                        # Z = tanh(bz + Uz x + Wz S)
                        z_t = [act_pool.tile([128, nb], f32, tag=f"z{m}",
                                             name=f"z_{t_u}_{i}_{m}")
                               for m in range(NM)]
                        for mq in (0, 4):
                            pss = {mq + c: ps_pool.tile(
                                [128, nb], f32, tag="ps",
                                name=f"ps_z_{t_u}_{i}_{mq + c}")
                                for c in range(4)}
                            for c in range(4):
                                m = mq + c
                                cc = c if pack_k16 else 0
                                nc.tensor.matmul(pss[m][:], u_ap(1, m, cc),
                                                 xt[32 * cc:32 * cc + KI, :],
                                                 start=True, stop=False,
                                                 tile_position=(32 * cc, 0))
                            for c in range(4):
                                m = mq + c
                                for k in range(NM):
                                    nc.tensor.matmul(
                                        pss[m][:], w_ap(wz_sb, k, m),
                                        s_cur[k][:],
                                        start=False, stop=(k == NM - 1))
                                nc.scalar.activation(z_t[m][:], pss[m][:],
                                                     Tanh, bias=b_ap(1, m))

                        # SR = S * R, in place into R's tiles
                        for k in range(NM):
                            nc.vector.tensor_mul(r_t[k][:], s_cur[k][:],
                                                 r_t[k][:])

                        # H = tanh(bh + Uh x + Wg (S*R))
                        h_t = [act_pool.tile([128, nb], mdt, tag=f"h{m}",
                                             name=f"h_{t_u}_{i}_{m}")
                               for m in range(NM)]
                        for mq in (0, 4):
                            pss = {mq + c: ps_pool.tile(
                                [128, nb], f32, tag="ps",
                                name=f"ps_h_{t_u}_{i}_{mq + c}")
                                for c in range(4)}
                            for c in range(4):
                                m = mq + c
                                cc = c if pack_k16 else 0
                                nc.tensor.matmul(pss[m][:], u_ap(4, m, cc),
                                                 xt[32 * cc:32 * cc + KI, :],
                                                 start=True, stop=False,
                                                 tile_position=(32 * cc, 0))
                            for c in range(4):
                                m = mq + c
                                for k in range(NM):
                                    nc.tensor.matmul(
                                        pss[m][:], w_ap(wg_sb, k, m),
                                        r_t[k][:],
                                        start=False, stop=(k == NM - 1))
                                nc.scalar.activation(h_t[m][:], pss[m][:],
                                                     Tanh, bias=b_ap(4, m))

                        if i == 0:
                            # deferred (1 - G), now that H's matmuls are in
                            # flight
                            for m in range(NM):
                                nc.vector.tensor_scalar(g_t[m][:], g_t[m][:],
                                                        -1.0, 1.0,
                                                        op0=mult, op1=add)

                        # output = (1-G)*H + Z*S  (h <- (1-G)*h; z <- z*s;
                        # h += z)
                        for m in range(NM):
                            nc.vector.tensor_mul(h_t[m][:], g_t[m][:],
                                                 h_t[m][:])
                            nc.vector.tensor_mul(z_t[m][:], z_t[m][:],
                                                 s_cur[m][:])
                            nc.vector.tensor_add(h_t[m][:], h_t[m][:],
                                                 z_t[m][:])

                        if i < N_LAYERS - 1:
                            s_new = [s_pool.tile([128, nb], mdt, tag=f"s{k}",
                                                 name=f"s_{t_u}_{i + 1}_{k}")
                                     for k in range(NM)]
                            for m in range(NM):
                                nc.scalar.activation(s_new[m][:], h_t[m][:],
                                                     Tanh)
                            s_cur = s_new

                    # y = out_w @ output + out_b, deferred into the next
                    # tile's S1/G phase
                    pend = (h_t, t, t_u)
                    if out_slot == "end":
                        emit_out(pend)
                        pend = None

            if pend is not None:
                emit_out(pend)

    nc.compile()
    return nc


def _get_nc(bc=BC, nb=NB, mm_dt=MM_DT):
    key = (bc, nb, mm_dt)
    if key not in _BUILD_CACHE:
        _BUILD_CACHE[key] = _build(bc, nb, mm_dt)
    return _BUILD_CACHE[key]


def _prep_inputs(x, Sw_w, Sw_b, Uz_w, Uz_b, Wz_w, Wz_b, Ug_w, Ug_b, Wg_w,
                 Wg_b, Ur_w, Ur_b, Uh_w, Uh_b, out_w, out_b):
    f = np.float32
    xT = np.ascontiguousarray(np.asarray(x, f).T)               # [16, B]
    WzT = np.ascontiguousarray(np.asarray(Wz_w, f).T)           # [H, H]
    WgT = np.ascontiguousarray(np.asarray(Wg_w, f).T)
    U16 = np.concatenate(
        [np.asarray(w, f).T for w in (Sw_w, Uz_w, Ug_w, Ur_w, Uh_w)],
        axis=1)                                                 # [16, 5H]
    U = np.zeros((128, 5 * H), f)
    for c in range(4):
        U[32 * c:32 * c + KI] = U16
    bias = np.zeros((128, 48), f)
    combos = [
        np.asarray(Sw_b, f),
        np.asarray(Uz_b, f) + np.asarray(Wz_b, f),
        np.asarray(Ug_b, f) + np.asarray(Wg_b, f),
        np.asarray(Ur_b, f) + np.asarray(Wg_b, f),
        np.asarray(Uh_b, f) + np.asarray(Wg_b, f),
    ]
    for g, b in enumerate(combos):
        bias[:, g * NM:(g + 1) * NM] = b.reshape(NM, 128).T
    bias[:, 40] = np.float32(np.asarray(out_b, f)[0])
    OW = np.ascontiguousarray(np.asarray(out_w, f).reshape(NM, 128).T)
    return xT, WzT, WgT, U, bias, OW


def kernel(**inputs):
    from concourse.bass_utils import run_bass_kernel_spmd

    nc = _get_nc()
    in_maps = _make_in_maps(inputs)
    res = run_bass_kernel_spmd(nc, in_maps, list(range(NCORES)))
    y = np.concatenate([res.results[c]["Y"] for c in range(NCORES)], axis=1)
    return np.ascontiguousarray(y.reshape(B_FULL, 1)).astype(np.float32)


def _make_in_maps(inputs):
    xT, WzT, WgT, U, bias, OW = _prep_inputs(**inputs)
    return [{
        "xT": np.ascontiguousarray(xT[:, c * BC:(c + 1) * BC]),
        "WzT": WzT, "WgT": WgT, "U": U, "BIAS": bias, "OW": OW,
    } for c in range(NCORES)]


def timed_run(inputs, iters=5, nc=None, pipeline=1):
    """Build a persistent jitted runner (so walrus compiles once), stage the
    inputs on-device, and time repeated executions. Returns (best_ns,
    all_ns, output)."""
    import time
    import jax
    from jax.sharding import Mesh, PartitionSpec, NamedSharding
    from jax.experimental.shard_map import shard_map
    from concourse import bass2jax, mybir

    bass2jax.install_neuronx_cc_hook()
    if nc is None:
        nc = _get_nc()
    in_maps = _make_in_maps(inputs)
    n_cores = NCORES

    partition_name = (nc.partition_id_tensor.name
                      if nc.partition_id_tensor else None)
    in_names, out_names, out_avals, zero_outs = [], [], [], []
    for alloc in nc.m.functions[0].allocations:
        if not isinstance(alloc, mybir.MemoryLocationSet):
            continue
        name = alloc.memorylocations[0].name
        if alloc.kind == "ExternalInput":
            if name != partition_name:
                in_names.append(name)
        elif alloc.kind == "ExternalOutput":
            shape = tuple(alloc.tensor_shape)
            dtype = mybir.dt.np(alloc.dtype)
            out_names.append(name)
            out_avals.append(jax.core.ShapedArray(shape, dtype))
            zero_outs.append(np.zeros(shape, dtype))
    n_params = len(in_names)
    n_outs = len(out_avals)
    all_in = list(in_names) + list(out_names)
    if partition_name is not None:
        all_in.append(partition_name)
    donate = tuple(range(n_params, n_params + n_outs))

    def _body(*args):
        operands = list(args)
        if partition_name is not None:
            operands.append(bass2jax.partition_id_tensor())
        outs = bass2jax._bass_exec_p.bind(
            *operands,
            out_avals=tuple(out_avals),
            in_names=tuple(all_in),
            out_names=tuple(out_names),
            lowering_input_output_aliases=(),
            sim_require_finite=True,
            sim_require_nnan=True,
            nc=nc,
        )
        return tuple(outs)

    devices = jax.devices()[:n_cores]
    mesh = Mesh(np.asarray(devices), ("core",))
    spec = PartitionSpec("core")
    sharded = jax.jit(
        shard_map(_body, mesh=mesh, in_specs=(spec,) * (n_params + n_outs),
                  out_specs=(spec,) * n_outs, check_rep=False),
        donate_argnums=donate, keep_unused=True)

    sharding = NamedSharding(mesh, spec)
    dev_in = [
        jax.device_put(
            np.concatenate([np.asarray(in_maps[c][n]) for c in range(n_cores)],
                           axis=0), sharding)
        for n in in_names
    ]
    def fresh_zeros():
        return [np.zeros((n_cores * z.shape[0], *z.shape[1:]), z.dtype)
                for z in zero_outs]

    # warmup (compiles)
    outs = sharded(*dev_in, *fresh_zeros())
    jax.block_until_ready(outs)

    state = {"outs": outs}

    def run_once(pipeline_n=pipeline):
        zss = [fresh_zeros() for _ in range(pipeline_n)]
        t0 = time.perf_counter()
        all_outs = [sharded(*dev_in, *zs) for zs in zss]
        jax.block_until_ready(all_outs)
        state["outs"] = all_outs[-1]
        return int((time.perf_counter() - t0) * 1e9 / pipeline_n)

    def get_y():
        y = np.asarray(state["outs"][out_names.index("Y")])  # [8, BC]
        return np.ascontiguousarray(
            y.reshape(1, B_FULL).reshape(B_FULL, 1)).astype(np.float32)

    if iters is None:
        return run_once, get_y

    times = [run_once() for _ in range(iters)]
    return min(times), times, get_y()



# revision 2
# speedup vs baseline: 1.2151x; 1.2151x over previous
"""DGMNet (dense MLP, 4 DGM layers) Trainium2 kernel.

Strategy: data-parallel over the batch dim (65536 rows -> 8 cores x 8192).
Inside each core, activations live feature-major in SBUF ([128 features x
batch-tile] tiles); every gate matmul is out[M=feat,N=batch] =
W.T-slice @ S with PE accumulation over the 1024-feature contraction, and
the x-side projections (K=16) are folded into the same PSUM accumulation
group (issued 4-wide via row tiling).

Two key wins over the fp32 formulation:
  1. fp16 matmul operands: the PE streams 16-bit operands at 1 col/cycle
     (2x the fp32/float32r byte-rate) and FWL halves weight-load time.
  2. Layer-0 algebraic fold: S1 = x@Sw.T+b is affine in the 16-wide x, so
     the G (wgS1), layer-0 Z and layer-0 R pre-activations are affine in
     x too. Host-side we fold Wg@Sw and Wz@Sw into 1024x16 matrices, so
     those three 1024-contraction matmuls become K=16 matmuls. Only 10 of
     the original 12 big matmuls per batch-tile remain.

Host-side preprocessing (numpy): transpose x/weights so the kernel never
transposes on-device, build the folded U block (7 gates x [16,1024]),
replicate it at partitions 0/32/64/96 for row-tiled K=16 matmuls, and
fold all bias pairs.
"""

import sys

sys.path.insert(0, "/opt/trn_rl_repo")

import numpy as np

B_FULL = 65536
KI = 16
H = 1024
NCORES = 8
BC = B_FULL // NCORES  # per-core batch (8192)
NB = 512               # batch tile (one PSUM bank of fp32)
NM = H // 128          # feature tiles (8)
N_LAYERS = 4

MM_DT = "float16"

# gate indices into the U block / bias table
G_S1, G_Z0, G_G, G_R0, G_Z, G_R, G_H = range(7)
NGATES = 7
B_OUT = NGATES * NM  # bias column holding out_b (56)

_BUILD_CACHE = {}


def _build(bc, nb, mm_dt, repeat=1):
    """Build + compile the single-core Bass program. Returns nc.

    repeat > 1 re-runs the whole computation (for slope-based timing of the
    device execution under the large axon dispatch overhead)."""
    import concourse.bacc as bacc
    import concourse.mybir as mybir
    import concourse.tile as tile

    f32 = mybir.dt.float32
    mdt = getattr(mybir.dt, mm_dt)
    Tanh = mybir.ActivationFunctionType.Tanh
    Ident = mybir.ActivationFunctionType.Identity
    mult = mybir.AluOpType.mult
    add = mybir.AluOpType.add

    nt = bc // nb

    nc = bacc.Bacc("TRN2", target_bir_lowering=False, debug=False,
                   num_devices=NCORES)

    xT_d = nc.dram_tensor("xT", [KI, bc], mdt, kind="ExternalInput").ap()
    wz_d = nc.dram_tensor("WzT", [H, H], mdt, kind="ExternalInput").ap()
    wg_d = nc.dram_tensor("WgT", [H, H], mdt, kind="ExternalInput").ap()
    u_d = nc.dram_tensor("U", [128, NGATES * H], mdt,
                         kind="ExternalInput").ap()
    bias_d = nc.dram_tensor("BIAS", [128, 64], f32, kind="ExternalInput").ap()
    ow_d = nc.dram_tensor("OW", [128, NM], mdt, kind="ExternalInput").ap()
    y_d = nc.dram_tensor("Y", [1, bc], f32, kind="ExternalOutput").ap()

    with tile.TileContext(nc) as tc:
        with (
            tc.tile_pool(name="const", bufs=1) as cpool,
            tc.tile_pool(name="xt", bufs=3) as xt_pool,
            tc.tile_pool(name="s", bufs=2) as s_pool,
            tc.tile_pool(name="act", bufs=1) as act_pool,
            tc.tile_pool(name="ov", bufs=2) as ov_pool,
            tc.tile_pool(name="psum", bufs=7, space="PSUM") as ps_pool,
            tc.tile_pool(name="pso", bufs=1, space="PSUM") as pso_pool,
        ):
            # ---- resident constants (small ones first; H needs Wg before
            # Z needs Wz, so load Wg ahead of Wz) -------------------------
            u_sb = cpool.tile([128, NGATES * H], mdt)
            nc.gpsimd.dma_start(u_sb[:], u_d[:])
            bias_sb = cpool.tile([128, 64], f32)
            nc.gpsimd.dma_start(bias_sb[:], bias_d[:])
            ow_sb = cpool.tile([128, NM], mdt)
            nc.gpsimd.dma_start(ow_sb[:], ow_d[:])
            wg_sb = cpool.tile([128, NM * H], mdt)
            wz_sb = cpool.tile([128, NM * H], mdt)
            for k in range(NM):
                nc.gpsimd.dma_start(wg_sb[:, k * H:(k + 1) * H],
                                    wg_d[k * 128:(k + 1) * 128, :])
            for k in range(NM):
                nc.gpsimd.dma_start(wz_sb[:, k * H:(k + 1) * H],
                                    wz_d[k * 128:(k + 1) * 128, :])

            def w_ap(w_sb, k, m):
                return w_sb[:, k * H + m * 128:k * H + (m + 1) * 128]

            def u_ap(g, m, c):
                return u_sb[32 * c:32 * c + KI,
                            g * H + m * 128:g * H + (m + 1) * 128]

            def b_ap(g, m):
                return bias_sb[:, g * NM + m:g * NM + m + 1]

            def x_starts(gate, xt, nametag, single):
                """Row-tiled (4-concurrent) K=16 start matmuls for m-quads.
                PSUM tiles are allocated per quad so the second quad never
                waits on pool slots freed by the first quad's evacuation."""
                pss = [None] * NM
                for mq in (0, 4):
                    for c in range(4):
                        m = mq + c
                        pss[m] = ps_pool.tile([128, nb], f32, tag="ps",
                                              name=f"{nametag}_{m}")
                    for c in range(4):
                        m = mq + c
                        nc.tensor.matmul(
                            pss[m][:], u_ap(gate, m, c),
                            xt[32 * c:32 * c + KI, :],
                            start=True, stop=single,
                            tile_position=(32 * c, 0))
                return pss

            def big_gate(gate, w_sb, xt, rhs, dest, act_fn, nametag):
                """Full gate: K=16 start into PSUM + 8 k-tile [128,128]
                matmuls against rhs, ACT evacuation with fused bias."""
                for mq in (0, 4):
                    pss = {}
                    for c in range(4):
                        m = mq + c
                        pss[m] = ps_pool.tile([128, nb], f32, tag="ps",
                                              name=f"{nametag}_{m}")
                    for c in range(4):
                        m = mq + c
                        nc.tensor.matmul(
                            pss[m][:], u_ap(gate, m, c),
                            xt[32 * c:32 * c + KI, :],
                            start=True, stop=False,
                            tile_position=(32 * c, 0))
                    for c in range(4):
                        m = mq + c
                        for k in range(NM):
                            nc.tensor.matmul(
                                pss[m][:], w_ap(w_sb, k, m), rhs[k][:],
                                start=False, stop=(k == NM - 1))
                        nc.scalar.activation(dest[m][:], pss[m][:], act_fn,
                                             bias=b_ap(gate, m))

            # ---- per batch tile -----------------------------------------
            pend = None  # deferred output row of the previous batch tile

            def emit_out(pend):
                h_prev, tp, up = pend
                po = pso_pool.tile([1, nb], f32, tag="po", name=f"po_{up}")
                for k in range(NM):
                    nc.tensor.matmul(po[:], ow_sb[:, k:k + 1], h_prev[k][:],
                                     start=(k == 0), stop=(k == NM - 1))
                orow = ov_pool.tile([1, nb], f32, tag="orow", name=f"orow_{up}")
                nc.vector.tensor_scalar_add(orow[:], po[:],
                                            bias_sb[0:1, B_OUT:B_OUT + 1])
                nc.gpsimd.dma_start(y_d[0:1, tp * nb:(tp + 1) * nb], orow[:])

            for rep in range(repeat):
                for t in range(nt):
                    t_u = rep * nt + t  # unique suffix for tile names
                    xt = xt_pool.tile([128, nb], mdt, tag="xt",
                                      name=f"xt_{t_u}")
                    for c in range(4):
                        nc.gpsimd.dma_start(xt[32 * c:32 * c + KI, :],
                                            xT_d[:, t * nb:(t + 1) * nb])

                    # S1 = x @ Sw.T + b (raw, no tanh) -- K=16 only
                    s_cur = [s_pool.tile([128, nb], mdt, tag=f"s{k}",
                                         name=f"s_{t_u}_0_{k}")
                             for k in range(NM)]
                    ps1 = x_starts(G_S1, xt, f"ps_s1_{t_u}", single=True)
                    for m in range(NM):
                        nc.scalar.activation(s_cur[m][:], ps1[m][:],
                                             Ident, bias=b_ap(G_S1, m))

                    # previous tile's output row fills the S1 dep gap
                    if pend is not None:
                        emit_out(pend)
                        pend = None

                    # G = tanh((Ug + Wg Sw) x + b') -- folded, K=16 only.
                    # Loop-invariant across layers; (1-G) deferred until
                    # H0's matmuls are in flight.
                    g_t = [act_pool.tile([128, nb], f32, tag=f"g{m}",
                                         name=f"g_{t_u}_{m}")
                           for m in range(NM)]
                    psg = x_starts(G_G, xt, f"ps_g_{t_u}", single=True)
                    for m in range(NM):
                        nc.scalar.activation(g_t[m][:], psg[m][:], Tanh,
                                             bias=b_ap(G_G, m))

                    # Z0 = tanh((Uz + Wz Sw) x + b') -- folded, K=16 only
                    z_t = [act_pool.tile([128, nb], f32, tag=f"z{m}",
                                         name=f"z_{t_u}_0_{m}")
                           for m in range(NM)]
                    psz = x_starts(G_Z0, xt, f"ps_z0_{t_u}", single=True)
                    for m in range(NM):
                        nc.scalar.activation(z_t[m][:], psz[m][:], Tanh,
                                             bias=b_ap(G_Z0, m))

                    # R0 = tanh((Ur + Wg Sw) x + b') -- folded, K=16 only
                    r_t = [act_pool.tile([128, nb], mdt, tag=f"r{m}",
                                         name=f"r_{t_u}_0_{m}")
                           for m in range(NM)]
                    psr = x_starts(G_R0, xt, f"ps_r0_{t_u}", single=True)
                    for m in range(NM):
                        nc.scalar.activation(r_t[m][:], psr[m][:], Tanh,
                                             bias=b_ap(G_R0, m))

                    for i in range(N_LAYERS):
                        if i > 0:
                            # R = tanh(br + Ur x + Wg S)
                            r_t = [act_pool.tile([128, nb], mdt, tag=f"r{m}",
                                                 name=f"r_{t_u}_{i}_{m}")
                                   for m in range(NM)]
                            big_gate(G_R, wg_sb, xt, s_cur, r_t, Tanh,
                                     f"ps_r_{t_u}_{i}")
                            # Z = tanh(bz + Uz x + Wz S)
                            z_t = [act_pool.tile([128, nb], f32, tag=f"z{m}",
                                                 name=f"z_{t_u}_{i}_{m}")
                                   for m in range(NM)]
                            big_gate(G_Z, wz_sb, xt, s_cur, z_t, Tanh,
                                     f"ps_z_{t_u}_{i}")

                        # SR = S * R, in place into R's tiles
                        for k in range(NM):
                            nc.vector.tensor_mul(r_t[k][:], s_cur[k][:],
                                                 r_t[k][:])

                        # H = tanh(bh + Uh x + Wg (S*R))
                        h_t = [act_pool.tile([128, nb], mdt, tag=f"h{m}",
                                             name=f"h_{t_u}_{i}_{m}")
                               for m in range(NM)]
                        big_gate(G_H, wg_sb, xt, r_t, h_t, Tanh,
                                 f"ps_h_{t_u}_{i}")

                        if i == 0:
                            # deferred (1 - G), now that H0's matmuls are in
                            # flight
                            for m in range(NM):
                                nc.vector.tensor_scalar(g_t[m][:], g_t[m][:],
                                                        -1.0, 1.0,
                                                        op0=mult, op1=add)

                        # output = (1-G)*H + Z*S  (h <- (1-G)*h; z <- z*s;
                        # h += z)
                        for m in range(NM):
                            nc.vector.tensor_mul(h_t[m][:], g_t[m][:],
                                                 h_t[m][:])
                            nc.vector.tensor_mul(z_t[m][:], z_t[m][:],
                                                 s_cur[m][:])
                            nc.vector.tensor_add(h_t[m][:], h_t[m][:],
                                                 z_t[m][:])

                        if i < N_LAYERS - 1:
                            s_new = [s_pool.tile([128, nb], mdt, tag=f"s{k}",
                                                 name=f"s_{t_u}_{i + 1}_{k}")
                                     for k in range(NM)]
                            for m in range(NM):
                                nc.scalar.activation(s_new[m][:], h_t[m][:],
                                                     Tanh)
                            s_cur = s_new

                    # y = out_w @ output + out_b, deferred into the next
                    # tile's S1 phase
                    pend = (h_t, t, t_u)

            if pend is not None:
                emit_out(pend)

    nc.compile()
    return nc


def _get_nc(bc=BC, nb=NB, mm_dt=MM_DT):
    key = (bc, nb, mm_dt)
    if key not in _BUILD_CACHE:
        _BUILD_CACHE[key] = _build(bc, nb, mm_dt)
    return _BUILD_CACHE[key]


def _prep_inputs(x, Sw_w, Sw_b, Uz_w, Uz_b, Wz_w, Wz_b, Ug_w, Ug_b, Wg_w,
                 Wg_b, Ur_w, Ur_b, Uh_w, Uh_b, out_w, out_b):
    f = np.float32
    h = np.float16
    Sw = np.asarray(Sw_w, f)
    Wz = np.asarray(Wz_w, f)
    Wg = np.asarray(Wg_w, f)
    WzSw = Wz @ Sw                                          # [H, 16]
    WgSw = Wg @ Sw
    xT = np.ascontiguousarray(np.asarray(x, f).T).astype(h)  # [16, B]
    WzT = np.ascontiguousarray(Wz.T).astype(h)               # [H, H]
    WgT = np.ascontiguousarray(Wg.T).astype(h)
    gates_U = [
        Sw,                                                  # S1
        np.asarray(Uz_w, f) + WzSw,                          # Z0 folded
        np.asarray(Ug_w, f) + WgSw,                          # G folded
        np.asarray(Ur_w, f) + WgSw,                          # R0 folded
        np.asarray(Uz_w, f),                                 # Z
        np.asarray(Ur_w, f),                                 # R
        np.asarray(Uh_w, f),                                 # H
    ]
    U16 = np.concatenate([w.T for w in gates_U], axis=1)     # [16, 7H]
    U = np.zeros((128, NGATES * H), h)
    for c in range(4):
        U[32 * c:32 * c + KI] = U16.astype(h)
    WzSb = Wz @ np.asarray(Sw_b, f)
    WgSb = Wg @ np.asarray(Sw_b, f)
    combos = [
        np.asarray(Sw_b, f),
        np.asarray(Uz_b, f) + np.asarray(Wz_b, f) + WzSb,
        np.asarray(Ug_b, f) + np.asarray(Wg_b, f) + WgSb,
        np.asarray(Ur_b, f) + np.asarray(Wg_b, f) + WgSb,
        np.asarray(Uz_b, f) + np.asarray(Wz_b, f),
        np.asarray(Ur_b, f) + np.asarray(Wg_b, f),
        np.asarray(Uh_b, f) + np.asarray(Wg_b, f),
    ]
    bias = np.zeros((128, 64), f)
    for g, b in enumerate(combos):
        bias[:, g * NM:(g + 1) * NM] = b.reshape(NM, 128).T
    bias[:, B_OUT] = np.float32(np.asarray(out_b, f)[0])
    OW = np.ascontiguousarray(
        np.asarray(out_w, f).reshape(NM, 128).T).astype(h)
    return xT, WzT, WgT, U, bias, OW


def kernel(**inputs):
    from concourse.bass_utils import run_bass_kernel_spmd

    nc = _get_nc()
    in_maps = _make_in_maps(inputs)
    res = run_bass_kernel_spmd(nc, in_maps, list(range(NCORES)))
    y = np.concatenate([res.results[c]["Y"] for c in range(NCORES)], axis=1)
    return np.ascontiguousarray(y.reshape(B_FULL, 1)).astype(np.float32)


def _make_in_maps(inputs):
    xT, WzT, WgT, U, bias, OW = _prep_inputs(**inputs)
    return [{
        "xT": np.ascontiguousarray(xT[:, c * BC:(c + 1) * BC]),
        "WzT": WzT, "WgT": WgT, "U": U, "BIAS": bias, "OW": OW,
    } for c in range(NCORES)]


def timed_run(inputs, iters=5, nc=None, pipeline=1):
    """Build a persistent jitted runner (so walrus compiles once), stage the
    inputs on-device, and time repeated executions. Returns (best_ns,
    all_ns, output)."""
    import time
    import jax
    from jax.sharding import Mesh, PartitionSpec, NamedSharding
    from jax.experimental.shard_map import shard_map
    from concourse import bass2jax, mybir

    bass2jax.install_neuronx_cc_hook()
    if nc is None:
        nc = _get_nc()
    in_maps = _make_in_maps(inputs)
    n_cores = NCORES

    partition_name = (nc.partition_id_tensor.name
                      if nc.partition_id_tensor else None)
    in_names, out_names, out_avals, zero_outs = [], [], [], []
    for alloc in nc.m.functions[0].allocations:
        if not isinstance(alloc, mybir.MemoryLocationSet):
            continue
        name = alloc.memorylocations[0].name
        if alloc.kind == "ExternalInput":
            if name != partition_name:
                in_names.append(name)
        elif alloc.kind == "ExternalOutput":
            shape = tuple(alloc.tensor_shape)
            dtype = mybir.dt.np(alloc.dtype)
            out_names.append(name)
            out_avals.append(jax.core.ShapedArray(shape, dtype))
            zero_outs.append(np.zeros(shape, dtype))
    n_params = len(in_names)
    n_outs = len(out_avals)
    all_in = list(in_names) + list(out_names)
    if partition_name is not None:
        all_in.append(partition_name)
    donate = tuple(range(n_params, n_params + n_outs))

    def _body(*args):
        operands = list(args)
        if partition_name is not None:
            operands.append(bass2jax.partition_id_tensor())
        outs = bass2jax._bass_exec_p.bind(
            *operands,
            out_avals=tuple(out_avals),
            in_names=tuple(all_in),
            out_names=tuple(out_names),
            lowering_input_output_aliases=(),
            sim_require_finite=True,
            sim_require_nnan=True,
            nc=nc,
        )
        return tuple(outs)

    devices = jax.devices()[:n_cores]
    mesh = Mesh(np.asarray(devices), ("core",))
    spec = PartitionSpec("core")
    sharded = jax.jit(
        shard_map(_body, mesh=mesh, in_specs=(spec,) * (n_params + n_outs),
                  out_specs=(spec,) * n_outs, check_rep=False),
        donate_argnums=donate, keep_unused=True)

    sharding = NamedSharding(mesh, spec)
    dev_in = [
        jax.device_put(
            np.concatenate([np.asarray(in_maps[c][n]) for c in range(n_cores)],
                           axis=0), sharding)
        for n in in_names
    ]
    def fresh_zeros():
        return [np.zeros((n_cores * z.shape[0], *z.shape[1:]), z.dtype)
                for z in zero_outs]

    # warmup (compiles)
    outs = sharded(*dev_in, *fresh_zeros())
    jax.block_until_ready(outs)

    state = {"outs": outs}

    def run_once(pipeline_n=pipeline):
        zss = [fresh_zeros() for _ in range(pipeline_n)]
        t0 = time.perf_counter()
        all_outs = [sharded(*dev_in, *zs) for zs in zss]
        jax.block_until_ready(all_outs)
        state["outs"] = all_outs[-1]
        return int((time.perf_counter() - t0) * 1e9 / pipeline_n)

    def get_y():
        y = np.asarray(state["outs"][out_names.index("Y")])  # [8, BC]
        return np.ascontiguousarray(
            y.reshape(1, B_FULL).reshape(B_FULL, 1)).astype(np.float32)

    if iters is None:
        return run_once, get_y

    times = [run_once() for _ in range(iters)]
    return min(times), times, get_y()


# revision 4
# speedup vs baseline: 1.5612x; 1.2848x over previous
"""DGMNet (dense MLP, 4 DGM layers) Trainium2 kernel.

Strategy: data-parallel over the batch dim (65536 rows -> 8 cores x 8192).
Inside each core, activations live feature-major in SBUF; every gate
matmul is out[M=feat,N=batch] = W.T-slice @ S with PE accumulation over
the 1024-feature contraction.

Speed levers over the fp32 formulation:
  1. Layer-0 algebraic fold: S1 = x@Sw.T+b is affine in the 16-wide x, so
     the G (wgS1), layer-0 Z and layer-0 R pre-activations are affine in
     x too. Host-side we fold Wg@Sw and Wz@Sw into 1024x16 matrices, so
     those three 1024-contraction matmuls become K=16 matmuls. Only 10 of
     the original 12 big matmuls per batch-tile remain.
  2. fp8e4m3 DoubleRow matmuls for H0 and all of layers 1-2 (7 of the 10
     big gates): 2 k-tiles per MM at ~108 ns per 512-col-MM-equivalent,
     2x the fp16 rate (measured). Layer 3 stays fp16: quantization errors
     add in quadrature and the final layer contributes the most, so this
     config sims at rel-err 1.6e-2 vs the 2e-2 budget.
  3. Biases ride the matmul via a 17th ones-row of x (so U blocks are
     [17, 1024] with the bias as row 16), letting each PSUM pair evacuate
     with ONE bias-free ACT tanh over [128, 2, 512] (halves ACT
     instruction count; ACT has a 352-cycle fixed cost per op).

Host-side preprocessing (numpy): transpose x/weights, build the folded U
block (7 gates x [17,1024] incl. bias row) replicated at partitions
0/32/64/96 for row-tiled K=17 matmuls, fp8-quantize Wg/Wz into
[128, 8, 1024] k-tile-major layout.
"""

import sys

sys.path.insert(0, "/opt/trn_rl_repo")

import numpy as np

B_FULL = 65536
KI = 16
KI1 = 17               # x rows + ones row for bias
H = 1024
NCORES = 8
BC = B_FULL // NCORES  # per-core batch (8192)
NB = 512               # batch tile (one PSUM bank of fp32)
NM = H // 128          # feature tiles (8)
NP = NM // 2           # feature-tile pairs (4)
N_LAYERS = 4

MM_DT = "float16"
FP8 = True             # fp8 DoubleRow for H0 + layers 1-2

# gate indices into the U block / bias table
G_S1, G_Z0, G_G, G_R0, G_Z, G_R, G_H = range(7)
NGATES = 7

_BUILD_CACHE = {}


def _build(bc, nb, mm_dt, repeat=1, fp8=FP8):
    """Build + compile the single-core Bass program. Returns nc.

    repeat > 1 re-runs the whole computation (for slope-based timing of the
    device execution under the large axon dispatch overhead)."""
    import concourse.bacc as bacc
    import concourse.mybir as mybir
    import concourse.tile as tile

    f32 = mybir.dt.float32
    mdt = getattr(mybir.dt, mm_dt)
    f8 = mybir.dt.float8e4
    DR = mybir.MatmulPerfMode.DoubleRow
    Tanh = mybir.ActivationFunctionType.Tanh
    mult = mybir.AluOpType.mult
    add = mybir.AluOpType.add

    nt = bc // nb

    nc = bacc.Bacc("TRN2", target_bir_lowering=False, debug=False,
                   num_devices=NCORES)

    xT_d = nc.dram_tensor("xT", [KI1, bc], mdt, kind="ExternalInput").ap()
    wz_d = nc.dram_tensor("WzT", [H, H], mdt, kind="ExternalInput").ap()
    wg_d = nc.dram_tensor("WgT", [H, H], mdt, kind="ExternalInput").ap()
    u_d = nc.dram_tensor("U", [128, NGATES * H], mdt,
                         kind="ExternalInput").ap()
    bias_d = nc.dram_tensor("BIAS", [1, 2], f32, kind="ExternalInput").ap()
    ow_d = nc.dram_tensor("OW", [128, NM], mdt, kind="ExternalInput").ap()
    if fp8:
        wg8_d = nc.dram_tensor("Wg8", [128, NM * H], f8,
                               kind="ExternalInput").ap()
        wz8_d = nc.dram_tensor("Wz8", [128, NM * H], f8,
                               kind="ExternalInput").ap()
    y_d = nc.dram_tensor("Y", [1, bc], f32, kind="ExternalOutput").ap()

    with tile.TileContext(nc) as tc:
        with (
            tc.tile_pool(name="const", bufs=1) as cpool,
            tc.tile_pool(name="xt", bufs=3) as xt_pool,
            tc.tile_pool(name="s", bufs=2) as s_pool,
            tc.tile_pool(name="act", bufs=1) as act_pool,
            tc.tile_pool(name="ov", bufs=2) as ov_pool,
            tc.tile_pool(name="psum", bufs=3, space="PSUM") as ps_pool,
            tc.tile_pool(name="pso", bufs=1, space="PSUM") as pso_pool,
        ):
            # ---- resident constants ------------------------------------
            u_sb = cpool.tile([128, NGATES * H], mdt)
            nc.gpsimd.dma_start(u_sb[:], u_d[:])
            bias_sb = cpool.tile([1, 2], f32)
            nc.gpsimd.dma_start(bias_sb[:], bias_d[:])
            ow_sb = cpool.tile([128, NM], mdt)
            nc.gpsimd.dma_start(ow_sb[:], ow_d[:])
            if fp8:
                wg8_sb = cpool.tile([128, NM, H], f8)
                nc.gpsimd.dma_start(wg8_sb[:, :, :], wg8_d[:])
                wz8_sb = cpool.tile([128, NM, H], f8)
                nc.gpsimd.dma_start(wz8_sb[:, :, :], wz8_d[:])
            wg_sb = cpool.tile([128, NM * H], mdt)
            wz_sb = cpool.tile([128, NM * H], mdt)
            for k in range(NM):
                nc.gpsimd.dma_start(wg_sb[:, k * H:(k + 1) * H],
                                    wg_d[k * 128:(k + 1) * 128, :])
            for k in range(NM):
                nc.gpsimd.dma_start(wz_sb[:, k * H:(k + 1) * H],
                                    wz_d[k * 128:(k + 1) * 128, :])

            def w_ap(w_sb, k, m):
                return w_sb[:, k * H + m * 128:k * H + (m + 1) * 128]

            def u_ap(g, m, c):
                return u_sb[32 * c:32 * c + KI1,
                            g * H + m * 128:g * H + (m + 1) * 128]

            def k17_quad(gate, xt, jq, single, nametag):
                """Two [128,2,nb] pair-PSUMs for j=jq,jq+1 with row-tiled
                K=17 start matmuls (bias rides row 16 of xt/U)."""
                pps = []
                for j in (jq, jq + 1):
                    pp = ps_pool.tile([128, 2, nb], f32, tag="ps",
                                      name=f"{nametag}_{j}")
                    pps.append(pp)
                for idx, j in enumerate((jq, jq + 1)):
                    for h2 in range(2):
                        m = 2 * j + h2
                        c = m % 4
                        nc.tensor.matmul(
                            pps[idx][:, h2:h2 + 1, :], u_ap(gate, m, c),
                            xt[32 * c:32 * c + KI1, :],
                            start=True, stop=single,
                            tile_position=(32 * c, 0))
                return pps

            def small_gate(gate, xt, dests, act, nametag):
                """K=17-only gate (folded): quad starts + pair evacuation."""
                for jq in (0, 2):
                    pps = k17_quad(gate, xt, jq, True, nametag)
                    for idx, j in enumerate((jq, jq + 1)):
                        if act is None:
                            nc.vector.tensor_copy(dests[j][:, :, :],
                                                  pps[idx][:, :, :])
                        else:
                            nc.scalar.activation(dests[j][:, :, :],
                                                 pps[idx][:, :, :], act)

            def big_gate8(gate, xt, w8, rhs8, dests, nametag):
                """fp8 DoubleRow gate: K=17 fp16 start + 4 DR matmuls (2
                k-tiles each) per m, pair-fused tanh evacuation."""
                for jq in (0, 2):
                    pps = k17_quad(gate, xt, jq, False, nametag)
                    for idx, j in enumerate((jq, jq + 1)):
                        for h2 in range(2):
                            m = 2 * j + h2
                            for kj in range(4):
                                nc.tensor.matmul(
                                    pps[idx][:, h2:h2 + 1, :],
                                    w8[:, 2 * kj:2 * kj + 2,
                                       m * 128:(m + 1) * 128],
                                    rhs8[:, 2 * kj:2 * kj + 2, :],
                                    start=False, stop=(kj == 3),
                                    perf_mode=DR)
                        nc.scalar.activation(dests[j][:, :, :],
                                             pps[idx][:, :, :], Tanh)

            def big_gate16(gate, xt, w_sb, rhs_pairs, dests, nametag):
                """fp16 gate: K=17 start + 8 k-tile matmuls per m."""
                for jq in (0, 2):
                    pps = k17_quad(gate, xt, jq, False, nametag)
                    for idx, j in enumerate((jq, jq + 1)):
                        for h2 in range(2):
                            m = 2 * j + h2
                            for k in range(NM):
                                nc.tensor.matmul(
                                    pps[idx][:, h2:h2 + 1, :],
                                    w_ap(w_sb, k, m),
                                    rhs_pairs[k // 2][:, k % 2:k % 2 + 1, :],
                                    start=False, stop=(k == NM - 1))
                        nc.scalar.activation(dests[j][:, :, :],
                                             pps[idx][:, :, :], Tanh)

            # ---- per batch tile -----------------------------------------
            pend = None  # deferred output row of the previous batch tile

            def emit_out(pend):
                h_prev, tp, up = pend
                po = pso_pool.tile([1, nb], f32, tag="po", name=f"po_{up}")
                for k in range(NM):
                    nc.tensor.matmul(po[:], ow_sb[:, k:k + 1],
                                     h_prev[k // 2][:, k % 2:k % 2 + 1, :],
                                     start=(k == 0), stop=(k == NM - 1))
                orow = ov_pool.tile([1, nb], f32, tag="orow", name=f"orow_{up}")
                nc.vector.tensor_scalar_add(orow[:], po[:],
                                            bias_sb[0:1, 0:1])
                nc.gpsimd.dma_start(y_d[0:1, tp * nb:(tp + 1) * nb], orow[:])

            def pair_tiles(tag, t_u, i, dt_):
                return [act_pool.tile([128, 2, nb], dt_, tag=f"{tag}{j}",
                                      name=f"{tag}_{t_u}_{i}_{j}")
                        for j in range(NP)]

            for rep in range(repeat):
                for t in range(nt):
                    t_u = rep * nt + t  # unique suffix for tile names
                    xt = xt_pool.tile([128, nb], mdt, tag="xt",
                                      name=f"xt_{t_u}")
                    for c in range(4):
                        nc.gpsimd.dma_start(xt[32 * c:32 * c + KI1, :],
                                            xT_d[:, t * nb:(t + 1) * nb])

                    # S1 = x @ Sw.T + b (raw; DVE copy evacuation)
                    s_cur = [s_pool.tile([128, 2, nb], mdt, tag=f"s{j}",
                                         name=f"s_{t_u}_0_{j}")
                             for j in range(NP)]
                    small_gate(G_S1, xt, s_cur, None, f"ps_s1_{t_u}")

                    # G = tanh((Ug + Wg Sw) x + b') -- folded, K=17 only.
                    # Loop-invariant across layers; (1-G) deferred until
                    # H0's matmuls are in flight.
                    g_t = pair_tiles("g", t_u, 0, mdt)
                    small_gate(G_G, xt, g_t, Tanh, f"ps_g_{t_u}")

                    # Z0 / R0: folded, K=17 only
                    z_t = pair_tiles("z", t_u, 0, mdt)
                    small_gate(G_Z0, xt, z_t, Tanh, f"ps_z0_{t_u}")
                    r_t = pair_tiles("r", t_u, 0, mdt)
                    small_gate(G_R0, xt, r_t, Tanh, f"ps_r0_{t_u}")

                    # previous tile's output row fills the dep gap
                    if pend is not None:
                        emit_out(pend)
                        pend = None

                    for i in range(N_LAYERS):
                        use8 = fp8 and i < N_LAYERS - 1
                        if i > 0:
                            r_t = pair_tiles("r", t_u, i, mdt)
                            z_t = pair_tiles("z", t_u, i, mdt)
                            if use8:
                                big_gate8(G_R, xt, wg8_sb, s8, r_t,
                                          f"ps_r_{t_u}_{i}")
                                big_gate8(G_Z, xt, wz8_sb, s8, z_t,
                                          f"ps_z_{t_u}_{i}")
                            else:
                                big_gate16(G_R, xt, wg_sb, s_cur, r_t,
                                           f"ps_r_{t_u}_{i}")
                                big_gate16(G_Z, xt, wz_sb, s_cur, z_t,
                                           f"ps_z_{t_u}_{i}")

                        # SR = S * R
                        h_t = pair_tiles("h", t_u, i, mdt)
                        if use8:
                            sr8 = act_pool.tile([128, NM, nb], f8,
                                                tag="sr8", bufs=2,
                                                name=f"sr8_{t_u}_{i}")
                            for j in range(NP):
                                nc.vector.tensor_mul(
                                    sr8[:, 2 * j:2 * j + 2, :],
                                    s_cur[j][:, :, :], r_t[j][:, :, :])
                            big_gate8(G_H, xt, wg8_sb, sr8, h_t,
                                      f"ps_h_{t_u}_{i}")
                        else:
                            for j in range(NP):
                                nc.vector.tensor_mul(r_t[j][:, :, :],
                                                     s_cur[j][:, :, :],
                                                     r_t[j][:, :, :])
                            big_gate16(G_H, xt, wg_sb, r_t, h_t,
                                       f"ps_h_{t_u}_{i}")

                        if i == 0:
                            # deferred (1 - G), now that H0's matmuls are
                            # in flight
                            for j in range(NP):
                                nc.vector.tensor_scalar(g_t[j][:, :, :],
                                                        g_t[j][:, :, :],
                                                        -1.0, 1.0,
                                                        op0=mult, op1=add)

                        # output = (1-G)*H + Z*S
                        for j in range(NP):
                            nc.vector.tensor_mul(h_t[j][:, :, :],
                                                 g_t[j][:, :, :],
                                                 h_t[j][:, :, :])
                            nc.vector.tensor_mul(z_t[j][:, :, :],
                                                 z_t[j][:, :, :],
                                                 s_cur[j][:, :, :])
                            nc.vector.tensor_add(h_t[j][:, :, :],
                                                 h_t[j][:, :, :],
                                                 z_t[j][:, :, :])

                        if i < N_LAYERS - 1:
                            s_new = [s_pool.tile([128, 2, nb], mdt,
                                                 tag=f"s{j}",
                                                 name=f"s_{t_u}_{i + 1}_{j}")
                                     for j in range(NP)]
                            for j in range(NP):
                                nc.scalar.activation(s_new[j][:, :, :],
                                                     h_t[j][:, :, :], Tanh)
                            if fp8 and i < N_LAYERS - 2:
                                # fp8 copy of S for next layer's R/Z rhs
                                s8 = act_pool.tile([128, NM, nb], f8,
                                                   tag="s8", bufs=2,
                                                   name=f"s8_{t_u}_{i + 1}")
                                for j in range(NP):
                                    nc.scalar.activation(
                                        s8[:, 2 * j:2 * j + 2, :],
                                        h_t[j][:, :, :], Tanh)
                            s_cur = s_new

                    # y = out_w @ output + out_b, deferred into the next
                    # tile's start phase
                    pend = (h_t, t, t_u)

            if pend is not None:
                emit_out(pend)

    nc.compile()
    return nc


def _get_nc(bc=BC, nb=NB, mm_dt=MM_DT):
    key = (bc, nb, mm_dt)
    if key not in _BUILD_CACHE:
        _BUILD_CACHE[key] = _build(bc, nb, mm_dt)
    return _BUILD_CACHE[key]


def _prep_inputs(x, Sw_w, Sw_b, Uz_w, Uz_b, Wz_w, Wz_b, Ug_w, Ug_b, Wg_w,
                 Wg_b, Ur_w, Ur_b, Uh_w, Uh_b, out_w, out_b):
    import ml_dtypes
    from concourse import mybir

    f = np.float32
    h = np.float16
    f8 = mybir.dt.np(mybir.dt.float8e4)
    Sw = np.asarray(Sw_w, f)
    Wz = np.asarray(Wz_w, f)
    Wg = np.asarray(Wg_w, f)
    WzSw = Wz @ Sw                                          # [H, 16]
    WgSw = Wg @ Sw
    xT = np.ones((KI1, B_FULL), h)
    xT[:KI] = np.asarray(x, f).T.astype(h)                  # row 16 stays 1.0
    WzT = np.ascontiguousarray(Wz.T).astype(h)              # [H, H]
    WgT = np.ascontiguousarray(Wg.T).astype(h)
    # fp8 copies in [128, k, H] k-tile-major layout
    Wg8 = np.ascontiguousarray(
        WgT.reshape(NM, 128, H).transpose(1, 0, 2).reshape(128, NM * H)
    ).astype(f8)
    Wz8 = np.ascontiguousarray(
        WzT.reshape(NM, 128, H).transpose(1, 0, 2).reshape(128, NM * H)
    ).astype(f8)
    WzSb = Wz @ np.asarray(Sw_b, f)
    WgSb = Wg @ np.asarray(Sw_b, f)
    gates_U = [
        (Sw, np.asarray(Sw_b, f)),                           # S1
        (np.asarray(Uz_w, f) + WzSw,
         np.asarray(Uz_b, f) + np.asarray(Wz_b, f) + WzSb),  # Z0 folded
        (np.asarray(Ug_w, f) + WgSw,
         np.asarray(Ug_b, f) + np.asarray(Wg_b, f) + WgSb),  # G folded
        (np.asarray(Ur_w, f) + WgSw,
         np.asarray(Ur_b, f) + np.asarray(Wg_b, f) + WgSb),  # R0 folded
        (np.asarray(Uz_w, f),
         np.asarray(Uz_b, f) + np.asarray(Wz_b, f)),         # Z
        (np.asarray(Ur_w, f),
         np.asarray(Ur_b, f) + np.asarray(Wg_b, f)),         # R
        (np.asarray(Uh_w, f),
         np.asarray(Uh_b, f) + np.asarray(Wg_b, f)),         # H
    ]
    U17 = np.concatenate(
        [np.concatenate([w.T, b.reshape(1, H)], axis=0) for w, b in gates_U],
        axis=1)                                              # [17, 7H]
    U = np.zeros((128, NGATES * H), h)
    for c in range(4):
        U[32 * c:32 * c + KI1] = U17.astype(h)
    bias = np.zeros((1, 2), f)
    bias[0, 0] = np.float32(np.asarray(out_b, f)[0])
    OW = np.ascontiguousarray(
        np.asarray(out_w, f).reshape(NM, 128).T).astype(h)
    return xT, WzT, WgT, U, bias, OW, Wg8, Wz8


def kernel(**inputs):
    from concourse.bass_utils import run_bass_kernel_spmd

    nc = _get_nc()
    in_maps = _make_in_maps(inputs)
    res = run_bass_kernel_spmd(nc, in_maps, list(range(NCORES)))
    y = np.concatenate([res.results[c]["Y"] for c in range(NCORES)], axis=1)
    return np.ascontiguousarray(y.reshape(B_FULL, 1)).astype(np.float32)


def _make_in_maps(inputs):
    xT, WzT, WgT, U, bias, OW, Wg8, Wz8 = _prep_inputs(**inputs)
    return [{
        "xT": np.ascontiguousarray(xT[:, c * BC:(c + 1) * BC]),
        "WzT": WzT, "WgT": WgT, "U": U, "BIAS": bias, "OW": OW,
        "Wg8": Wg8, "Wz8": Wz8,
    } for c in range(NCORES)]


def timed_run(inputs, iters=5, nc=None, pipeline=1):
    """Build a persistent jitted runner (so walrus compiles once), stage the
    inputs on-device, and time repeated executions. Returns (best_ns,
    all_ns, output)."""
    import time
    import jax
    from jax.sharding import Mesh, PartitionSpec, NamedSharding
    from jax.experimental.shard_map import shard_map
    from concourse import bass2jax, mybir

    bass2jax.install_neuronx_cc_hook()
    if nc is None:
        nc = _get_nc()
    in_maps = _make_in_maps(inputs)
    n_cores = NCORES

    partition_name = (nc.partition_id_tensor.name
                      if nc.partition_id_tensor else None)
    in_names, out_names, out_avals, zero_outs = [], [], [], []
    for alloc in nc.m.functions[0].allocations:
        if not isinstance(alloc, mybir.MemoryLocationSet):
            continue
        name = alloc.memorylocations[0].name
        if alloc.kind == "ExternalInput":
            if name != partition_name:
                in_names.append(name)
        elif alloc.kind == "ExternalOutput":
            shape = tuple(alloc.tensor_shape)
            dtype = mybir.dt.np(alloc.dtype)
            out_names.append(name)
            out_avals.append(jax.core.ShapedArray(shape, dtype))
            zero_outs.append(np.zeros(shape, dtype))
    n_params = len(in_names)
    n_outs = len(out_avals)
    all_in = list(in_names) + list(out_names)
    if partition_name is not None:
        all_in.append(partition_name)
    donate = tuple(range(n_params, n_params + n_outs))

    def _body(*args):
        operands = list(args)
        if partition_name is not None:
            operands.append(bass2jax.partition_id_tensor())
        outs = bass2jax._bass_exec_p.bind(
            *operands,
            out_avals=tuple(out_avals),
            in_names=tuple(all_in),
            out_names=tuple(out_names),
            lowering_input_output_aliases=(),
            sim_require_finite=True,
            sim_require_nnan=True,
            nc=nc,
        )
        return tuple(outs)

    devices = jax.devices()[:n_cores]
    mesh = Mesh(np.asarray(devices), ("core",))
    spec = PartitionSpec("core")
    sharded = jax.jit(
        shard_map(_body, mesh=mesh, in_specs=(spec,) * (n_params + n_outs),
                  out_specs=(spec,) * n_outs, check_rep=False),
        donate_argnums=donate, keep_unused=True)

    sharding = NamedSharding(mesh, spec)
    dev_in = [
        jax.device_put(
            np.concatenate([np.asarray(in_maps[c][n]) for c in range(n_cores)],
                           axis=0), sharding)
        for n in in_names
    ]
    def fresh_zeros():
        return [np.zeros((n_cores * z.shape[0], *z.shape[1:]), z.dtype)
                for z in zero_outs]

    # warmup (compiles)
    outs = sharded(*dev_in, *fresh_zeros())
    jax.block_until_ready(outs)

    state = {"outs": outs}

    def run_once(pipeline_n=pipeline):
        zss = [fresh_zeros() for _ in range(pipeline_n)]
        t0 = time.perf_counter()
        all_outs = [sharded(*dev_in, *zs) for zs in zss]
        jax.block_until_ready(all_outs)
        state["outs"] = all_outs[-1]
        return int((time.perf_counter() - t0) * 1e9 / pipeline_n)

    def get_y():
        y = np.asarray(state["outs"][out_names.index("Y")])  # [8, BC]
        return np.ascontiguousarray(
            y.reshape(1, B_FULL).reshape(B_FULL, 1)).astype(np.float32)

    if iters is None:
        return run_once, get_y

    times = [run_once() for _ in range(iters)]
    return min(times), times, get_y()


# revision 10
# speedup vs baseline: 1.5954x; 1.0219x over previous
"""DGMNet (dense MLP, 4 DGM layers) Trainium2 kernel.

Strategy: data-parallel over the batch dim (65536 rows -> 8 cores x 8192).
Inside each core, activations live feature-major in SBUF; every gate
matmul is out[M=feat,N=batch] = W.T-slice @ S with PE accumulation over
the 1024-feature contraction.

Speed levers over the fp32 formulation:
  1. Layer-0 algebraic fold: S1 = x@Sw.T+b is affine in the 16-wide x, so
     the G (wgS1), layer-0 Z and layer-0 R pre-activations are affine in
     x too. Host-side we fold Wg@Sw and Wz@Sw into 1024x16 matrices, so
     those three 1024-contraction matmuls become K=16 matmuls. Only 10 of
     the original 12 big matmuls per batch-tile remain.
  2. fp8e4m3 DoubleRow matmuls for H0 and all of layers 1-2 (7 of the 10
     big gates): 2 k-tiles per MM at ~108 ns per 512-col-MM-equivalent,
     2x the fp16 rate (measured). Layer 3 stays fp16: quantization errors
     add in quadrature and the final layer contributes the most, so this
     config sims at rel-err 1.6e-2 vs the 2e-2 budget.
  3. Biases ride the matmul via a 17th ones-row of x (so U blocks are
     [17, 1024] with the bias as row 16), letting each PSUM pair evacuate
     with ONE bias-free ACT tanh over [128, 2, 512] (halves ACT
     instruction count; ACT has a 352-cycle fixed cost per op).

Host-side preprocessing (numpy): transpose x/weights, build the folded U
block (7 gates x [17,1024] incl. bias row) replicated at partitions
0/32/64/96 for row-tiled K=17 matmuls, fp8-quantize Wg/Wz into
[128, 8, 1024] k-tile-major layout.
"""

import sys

sys.path.insert(0, "/opt/trn_rl_repo")

import numpy as np

B_FULL = 65536
KI = 16
KI1 = 17               # x rows + ones row for bias
H = 1024
NCORES = 8
BC = B_FULL // NCORES  # per-core batch (8192)
NB = 512               # batch tile (one PSUM bank of fp32)
NM = H // 128          # feature tiles (8)
NP = NM // 2           # feature-tile pairs (4)
N_LAYERS = 4

MM_DT = "float16"
FP8 = True             # fp8 DoubleRow for H0 + layers 1-2
FP8_Z3 = False         # additionally run layer-3 Z in fp8 (err 1.77e-2)

# gate indices into the U block / bias table
G_S1, G_Z0, G_G, G_R0, G_Z, G_R, G_H = range(7)
NGATES = 7

_BUILD_CACHE = {}


def _build(bc, nb, mm_dt, repeat=1, fp8=FP8):
    """Build + compile the single-core Bass program. Returns nc.

    repeat > 1 re-runs the whole computation (for slope-based timing of the
    device execution under the large axon dispatch overhead)."""
    import concourse.bacc as bacc
    import concourse.mybir as mybir
    import concourse.tile as tile

    f32 = mybir.dt.float32
    mdt = getattr(mybir.dt, mm_dt)
    f8 = mybir.dt.float8e4
    DR = mybir.MatmulPerfMode.DoubleRow
    Tanh = mybir.ActivationFunctionType.Tanh
    mult = mybir.AluOpType.mult
    add = mybir.AluOpType.add

    nt = bc // nb

    nc = bacc.Bacc("TRN2", target_bir_lowering=False, debug=False,
                   num_devices=NCORES)

    xT_d = nc.dram_tensor("xT", [KI1, bc], mdt, kind="ExternalInput").ap()
    wz_d = nc.dram_tensor("WzT", [H, H], mdt, kind="ExternalInput").ap()
    wg_d = nc.dram_tensor("WgT", [H, H], mdt, kind="ExternalInput").ap()
    u_d = nc.dram_tensor("U", [128, NGATES * H], mdt,
                         kind="ExternalInput").ap()
    bias_d = nc.dram_tensor("BIAS", [1, 2], f32, kind="ExternalInput").ap()
    ow_d = nc.dram_tensor("OW", [128, NM], mdt, kind="ExternalInput").ap()
    if fp8:
        wg8_d = nc.dram_tensor("Wg8", [128, NM * H], f8,
                               kind="ExternalInput").ap()
        wz8_d = nc.dram_tensor("Wz8", [128, NM * H], f8,
                               kind="ExternalInput").ap()
    y_d = nc.dram_tensor("Y", [1, bc], f32, kind="ExternalOutput").ap()

    with tile.TileContext(nc) as tc:
        with (
            tc.tile_pool(name="const", bufs=1) as cpool,
            tc.tile_pool(name="xt", bufs=3) as xt_pool,
            tc.tile_pool(name="s", bufs=2) as s_pool,
            tc.tile_pool(name="act", bufs=1) as act_pool,
            tc.tile_pool(name="ov", bufs=2) as ov_pool,
            tc.tile_pool(name="psum", bufs=3, space="PSUM") as ps_pool,
            tc.tile_pool(name="pso", bufs=1, space="PSUM") as pso_pool,
        ):
            # ---- resident constants ------------------------------------
            u_sb = cpool.tile([128, NGATES * H], mdt)
            nc.gpsimd.dma_start(u_sb[:], u_d[:])
            bias_sb = cpool.tile([1, 2], f32)
            nc.gpsimd.dma_start(bias_sb[:], bias_d[:])
            ow_sb = cpool.tile([128, NM], mdt)
            nc.gpsimd.dma_start(ow_sb[:], ow_d[:])
            if fp8:
                wg8_sb = cpool.tile([128, NM, H], f8)
                nc.gpsimd.dma_start(wg8_sb[:, :, :], wg8_d[:])
                wz8_sb = cpool.tile([128, NM, H], f8)
                nc.gpsimd.dma_start(wz8_sb[:, :, :], wz8_d[:])
            wg_sb = cpool.tile([128, NM * H], mdt)
            wz_sb = cpool.tile([128, NM * H], mdt)
            for k in range(NM):
                nc.gpsimd.dma_start(wg_sb[:, k * H:(k + 1) * H],
                                    wg_d[k * 128:(k + 1) * 128, :])
            for k in range(NM):
                nc.gpsimd.dma_start(wz_sb[:, k * H:(k + 1) * H],
                                    wz_d[k * 128:(k + 1) * 128, :])

            def w_ap(w_sb, k, m):
                return w_sb[:, k * H + m * 128:k * H + (m + 1) * 128]

            def u_ap(g, m, c):
                return u_sb[32 * c:32 * c + KI1,
                            g * H + m * 128:g * H + (m + 1) * 128]

            def k17_quad(gate, xt, jq, single, nametag):
                """Two [128,2,nb] pair-PSUMs for j=jq,jq+1 with row-tiled
                K=17 start matmuls (bias rides row 16 of xt/U)."""
                pps = []
                for j in (jq, jq + 1):
                    pp = ps_pool.tile([128, 2, nb], f32, tag="ps",
                                      name=f"{nametag}_{j}")
                    pps.append(pp)
                for idx, j in enumerate((jq, jq + 1)):
                    for h2 in range(2):
                        m = 2 * j + h2
                        c = m % 4
                        nc.tensor.matmul(
                            pps[idx][:, h2:h2 + 1, :], u_ap(gate, m, c),
                            xt[32 * c:32 * c + KI1, :],
                            start=True, stop=single,
                            tile_position=(32 * c, 0))
                return pps

            def small_gate(gate, xt, dests, act, nametag):
                """K=17-only gate (folded): quad starts + pair evacuation."""
                for jq in (0, 2):
                    pps = k17_quad(gate, xt, jq, True, nametag)
                    for idx, j in enumerate((jq, jq + 1)):
                        if act is None:
                            nc.vector.tensor_copy(dests[j][:, :, :],
                                                  pps[idx][:, :, :])
                        else:
                            nc.scalar.activation(dests[j][:, :, :],
                                                 pps[idx][:, :, :], act)

            def big_gate8(gate, xt, w8, rhs8, dests, nametag):
                """fp8 DoubleRow gate: K=17 fp16 start + 4 DR matmuls (2
                k-tiles each) per m, pair-fused tanh evacuation."""
                for jq in (0, 2):
                    pps = k17_quad(gate, xt, jq, False, nametag)
                    for idx, j in enumerate((jq, jq + 1)):
                        for h2 in range(2):
                            m = 2 * j + h2
                            for kj in range(4):
                                nc.tensor.matmul(
                                    pps[idx][:, h2:h2 + 1, :],
                                    w8[:, 2 * kj:2 * kj + 2,
                                       m * 128:(m + 1) * 128],
                                    rhs8[:, 2 * kj:2 * kj + 2, :],
                                    start=False, stop=(kj == 3),
                                    perf_mode=DR)
                        nc.scalar.activation(dests[j][:, :, :],
                                             pps[idx][:, :, :], Tanh)

            def big_gate16(gate, xt, w_sb, rhs_pairs, dests, nametag):
                """fp16 gate: K=17 start + 8 k-tile matmuls per m."""
                for jq in (0, 2):
                    pps = k17_quad(gate, xt, jq, False, nametag)
                    for idx, j in enumerate((jq, jq + 1)):
                        for h2 in range(2):
                            m = 2 * j + h2
                            for k in range(NM):
                                nc.tensor.matmul(
                                    pps[idx][:, h2:h2 + 1, :],
                                    w_ap(w_sb, k, m),
                                    rhs_pairs[k // 2][:, k % 2:k % 2 + 1, :],
                                    start=False, stop=(k == NM - 1))
                        nc.scalar.activation(dests[j][:, :, :],
                                             pps[idx][:, :, :], Tanh)

            # ---- per batch tile -----------------------------------------
            pend = None  # deferred output row of the previous batch tile

            def emit_out(pend):
                h_prev, tp, up = pend
                po = pso_pool.tile([1, nb], f32, tag="po", name=f"po_{up}")
                for k in range(NM):
                    nc.tensor.matmul(po[:], ow_sb[:, k:k + 1],
                                     h_prev[k // 2][:, k % 2:k % 2 + 1, :],
                                     start=(k == 0), stop=(k == NM - 1))
                orow = ov_pool.tile([1, nb], f32, tag="orow", name=f"orow_{up}")
                nc.vector.tensor_scalar_add(orow[:], po[:],
                                            bias_sb[0:1, 0:1])
                nc.gpsimd.dma_start(y_d[0:1, tp * nb:(tp + 1) * nb], orow[:])

            def pair_tiles(tag, t_u, i, dt_):
                return [act_pool.tile([128, 2, nb], dt_, tag=f"{tag}{j}",
                                      name=f"{tag}_{t_u}_{i}_{j}")
                        for j in range(NP)]

            for rep in range(repeat):
                for t in range(nt):
                    t_u = rep * nt + t  # unique suffix for tile names
                    xt = xt_pool.tile([128, nb], mdt, tag="xt",
                                      name=f"xt_{t_u}")
                    for c in range(4):
                        nc.gpsimd.dma_start(xt[32 * c:32 * c + KI1, :],
                                            xT_d[:, t * nb:(t + 1) * nb])

                    # S1 = x @ Sw.T + b (raw; DVE copy evacuation)
                    s_cur = [s_pool.tile([128, 2, nb], mdt, tag=f"s{j}",
                                         name=f"s_{t_u}_0_{j}")
                             for j in range(NP)]
                    small_gate(G_S1, xt, s_cur, None, f"ps_s1_{t_u}")

                    # G = tanh((Ug + Wg Sw) x + b') -- folded, K=17 only.
                    # Loop-invariant across layers; (1-G) deferred until
                    # H0's matmuls are in flight.
                    g_t = pair_tiles("g", t_u, 0, mdt)
                    small_gate(G_G, xt, g_t, Tanh, f"ps_g_{t_u}")

                    # Z0 / R0: folded, K=17 only
                    z_t = pair_tiles("z", t_u, 0, mdt)
                    small_gate(G_Z0, xt, z_t, Tanh, f"ps_z0_{t_u}")
                    r_t = pair_tiles("r", t_u, 0, mdt)
                    small_gate(G_R0, xt, r_t, Tanh, f"ps_r0_{t_u}")
                    # Z*S computed early (off the post-H critical chain)
                    for j in range(NP):
                        nc.vector.tensor_mul(z_t[j][:, :, :], z_t[j][:, :, :],
                                             s_cur[j][:, :, :])

                    # previous tile's output row fills the dep gap
                    if pend is not None:
                        emit_out(pend)
                        pend = None

                    for i in range(N_LAYERS):
                        use8 = fp8 and i < N_LAYERS - 1
                        if i > 0:
                            r_t = pair_tiles("r", t_u, i, mdt)
                            z_t = pair_tiles("z", t_u, i, mdt)
                            if use8:
                                big_gate8(G_R, xt, wg8_sb, s8, r_t,
                                          f"ps_r_{t_u}_{i}")
                                big_gate8(G_Z, xt, wz8_sb, s8, z_t,
                                          f"ps_z_{t_u}_{i}")
                            else:
                                big_gate16(G_R, xt, wg_sb, s_cur, r_t,
                                           f"ps_r_{t_u}_{i}")
                                if fp8 and FP8_Z3:
                                    big_gate8(G_Z, xt, wz8_sb, s8, z_t,
                                              f"ps_z_{t_u}_{i}")
                                else:
                                    big_gate16(G_Z, xt, wz_sb, s_cur, z_t,
                                               f"ps_z_{t_u}_{i}")
                            # Z*S early (off the post-H critical chain)
                            for j in range(NP):
                                nc.vector.tensor_mul(z_t[j][:, :, :],
                                                     z_t[j][:, :, :],
                                                     s_cur[j][:, :, :])

                        # SR = S * R
                        h_t = pair_tiles("h", t_u, i, mdt)
                        if use8:
                            sr8 = act_pool.tile([128, NM, nb], f8,
                                                tag="sr8", bufs=2,
                                                name=f"sr8_{t_u}_{i}")
                            for j in range(NP):
                                nc.vector.tensor_mul(
                                    sr8[:, 2 * j:2 * j + 2, :],
                                    s_cur[j][:, :, :], r_t[j][:, :, :])
                            big_gate8(G_H, xt, wg8_sb, sr8, h_t,
                                      f"ps_h_{t_u}_{i}")
                        else:
                            for j in range(NP):
                                nc.vector.tensor_mul(r_t[j][:, :, :],
                                                     s_cur[j][:, :, :],
                                                     r_t[j][:, :, :])
                            big_gate16(G_H, xt, wg_sb, r_t, h_t,
                                       f"ps_h_{t_u}_{i}")

                        if i == 0:
                            # deferred (1 - G), now that H0's matmuls are
                            # in flight
                            for j in range(NP):
                                nc.vector.tensor_scalar(g_t[j][:, :, :],
                                                        g_t[j][:, :, :],
                                                        -1.0, 1.0,
                                                        op0=mult, op1=add)

                        # output = (1-G)*H + Z*S  (Z*S already in z_t)
                        for j in range(NP):
                            nc.vector.tensor_mul(h_t[j][:, :, :],
                                                 g_t[j][:, :, :],
                                                 h_t[j][:, :, :])
                            nc.vector.tensor_add(h_t[j][:, :, :],
                                                 h_t[j][:, :, :],
                                                 z_t[j][:, :, :])

                        if i < N_LAYERS - 1:
                            s_new = [s_pool.tile([128, 2, nb], mdt,
                                                 tag=f"s{j}",
                                                 name=f"s_{t_u}_{i + 1}_{j}")
                                     for j in range(NP)]
                            for j in range(NP):
                                nc.scalar.activation(s_new[j][:, :, :],
                                                     h_t[j][:, :, :], Tanh)
                            if fp8 and (i < N_LAYERS - 2 or FP8_Z3):
                                # fp8 copy of S for next layer's R/Z rhs
                                s8 = act_pool.tile([128, NM, nb], f8,
                                                   tag="s8", bufs=2,
                                                   name=f"s8_{t_u}_{i + 1}")
                                for j in range(NP):
                                    nc.scalar.activation(
                                        s8[:, 2 * j:2 * j + 2, :],
                                        h_t[j][:, :, :], Tanh)
                            s_cur = s_new

                    # y = out_w @ output + out_b, deferred into the next
                    # tile's start phase
                    pend = (h_t, t, t_u)

            if pend is not None:
                emit_out(pend)

    nc.compile()
    return nc


def _get_nc(bc=BC, nb=NB, mm_dt=MM_DT):
    key = (bc, nb, mm_dt)
    if key not in _BUILD_CACHE:
        _BUILD_CACHE[key] = _build(bc, nb, mm_dt)
    return _BUILD_CACHE[key]


def _prep_inputs(x, Sw_w, Sw_b, Uz_w, Uz_b, Wz_w, Wz_b, Ug_w, Ug_b, Wg_w,
                 Wg_b, Ur_w, Ur_b, Uh_w, Uh_b, out_w, out_b):
    import ml_dtypes
    from concourse import mybir

    f = np.float32
    h = np.float16
    f8 = mybir.dt.np(mybir.dt.float8e4)
    Sw = np.asarray(Sw_w, f)
    Wz = np.asarray(Wz_w, f)
    Wg = np.asarray(Wg_w, f)
    WzSw = Wz @ Sw                                          # [H, 16]
    WgSw = Wg @ Sw
    xT = np.ones((KI1, B_FULL), h)
    xT[:KI] = np.asarray(x, f).T.astype(h)                  # row 16 stays 1.0
    WzT = np.ascontiguousarray(Wz.T).astype(h)              # [H, H]
    WgT = np.ascontiguousarray(Wg.T).astype(h)
    # fp8 copies in [128, k, H] k-tile-major layout
    Wg8 = np.ascontiguousarray(
        WgT.reshape(NM, 128, H).transpose(1, 0, 2).reshape(128, NM * H)
    ).astype(f8)
    Wz8 = np.ascontiguousarray(
        WzT.reshape(NM, 128, H).transpose(1, 0, 2).reshape(128, NM * H)
    ).astype(f8)
    WzSb = Wz @ np.asarray(Sw_b, f)
    WgSb = Wg @ np.asarray(Sw_b, f)
    gates_U = [
        (Sw, np.asarray(Sw_b, f)),                           # S1
        (np.asarray(Uz_w, f) + WzSw,
         np.asarray(Uz_b, f) + np.asarray(Wz_b, f) + WzSb),  # Z0 folded
        (np.asarray(Ug_w, f) + WgSw,
         np.asarray(Ug_b, f) + np.asarray(Wg_b, f) + WgSb),  # G folded
        (np.asarray(Ur_w, f) + WgSw,
         np.asarray(Ur_b, f) + np.asarray(Wg_b, f) + WgSb),  # R0 folded
        (np.asarray(Uz_w, f),
         np.asarray(Uz_b, f) + np.asarray(Wz_b, f)),         # Z
        (np.asarray(Ur_w, f),
         np.asarray(Ur_b, f) + np.asarray(Wg_b, f)),         # R
        (np.asarray(Uh_w, f),
         np.asarray(Uh_b, f) + np.asarray(Wg_b, f)),         # H
    ]
    U17 = np.concatenate(
        [np.concatenate([w.T, b.reshape(1, H)], axis=0) for w, b in gates_U],
        axis=1)                                              # [17, 7H]
    U = np.zeros((128, NGATES * H), h)
    for c in range(4):
        U[32 * c:32 * c + KI1] = U17.astype(h)
    bias = np.zeros((1, 2), f)
    bias[0, 0] = np.float32(np.asarray(out_b, f)[0])
    OW = np.ascontiguousarray(
        np.asarray(out_w, f).reshape(NM, 128).T).astype(h)
    return xT, WzT, WgT, U, bias, OW, Wg8, Wz8


def kernel(**inputs):
    from concourse.bass_utils import run_bass_kernel_spmd

    nc = _get_nc()
    in_maps = _make_in_maps(inputs)
    res = run_bass_kernel_spmd(nc, in_maps, list(range(NCORES)))
    y = np.concatenate([res.results[c]["Y"] for c in range(NCORES)], axis=1)
    return np.ascontiguousarray(y.reshape(B_FULL, 1)).astype(np.float32)


def _make_in_maps(inputs):
    xT, WzT, WgT, U, bias, OW, Wg8, Wz8 = _prep_inputs(**inputs)
    return [{
        "xT": np.ascontiguousarray(xT[:, c * BC:(c + 1) * BC]),
        "WzT": WzT, "WgT": WgT, "U": U, "BIAS": bias, "OW": OW,
        "Wg8": Wg8, "Wz8": Wz8,
    } for c in range(NCORES)]


def timed_run(inputs, iters=5, nc=None, pipeline=1):
    """Build a persistent jitted runner (so walrus compiles once), stage the
    inputs on-device, and time repeated executions. Returns (best_ns,
    all_ns, output)."""
    import time
    import jax
    from jax.sharding import Mesh, PartitionSpec, NamedSharding
    from jax.experimental.shard_map import shard_map
    from concourse import bass2jax, mybir

    bass2jax.install_neuronx_cc_hook()
    if nc is None:
        nc = _get_nc()
    in_maps = _make_in_maps(inputs)
    n_cores = NCORES

    partition_name = (nc.partition_id_tensor.name
                      if nc.partition_id_tensor else None)
    in_names, out_names, out_avals, zero_outs = [], [], [], []
    for alloc in nc.m.functions[0].allocations:
        if not isinstance(alloc, mybir.MemoryLocationSet):
            continue
        name = alloc.memorylocations[0].name
        if alloc.kind == "ExternalInput":
            if name != partition_name:
                in_names.append(name)
        elif alloc.kind == "ExternalOutput":
            shape = tuple(alloc.tensor_shape)
            dtype = mybir.dt.np(alloc.dtype)
            out_names.append(name)
            out_avals.append(jax.core.ShapedArray(shape, dtype))
            zero_outs.append(np.zeros(shape, dtype))
    n_params = len(in_names)
    n_outs = len(out_avals)
    all_in = list(in_names) + list(out_names)
    if partition_name is not None:
        all_in.append(partition_name)
    donate = tuple(range(n_params, n_params + n_outs))

    def _body(*args):
        operands = list(args)
        if partition_name is not None:
            operands.append(bass2jax.partition_id_tensor())
        outs = bass2jax._bass_exec_p.bind(
            *operands,
            out_avals=tuple(out_avals),
            in_names=tuple(all_in),
            out_names=tuple(out_names),
            lowering_input_output_aliases=(),
            sim_require_finite=True,
            sim_require_nnan=True,
            nc=nc,
        )
        return tuple(outs)

    devices = jax.devices()[:n_cores]
    mesh = Mesh(np.asarray(devices), ("core",))
    spec = PartitionSpec("core")
    sharded = jax.jit(
        shard_map(_body, mesh=mesh, in_specs=(spec,) * (n_params + n_outs),
                  out_specs=(spec,) * n_outs, check_rep=False),
        donate_argnums=donate, keep_unused=True)

    sharding = NamedSharding(mesh, spec)
    dev_in = [
        jax.device_put(
            np.concatenate([np.asarray(in_maps[c][n]) for c in range(n_cores)],
                           axis=0), sharding)
        for n in in_names
    ]
    def fresh_zeros():
        return [np.zeros((n_cores * z.shape[0], *z.shape[1:]), z.dtype)
                for z in zero_outs]

    # warmup (compiles)
    outs = sharded(*dev_in, *fresh_zeros())
    jax.block_until_ready(outs)

    state = {"outs": outs}

    def run_once(pipeline_n=pipeline):
        zss = [fresh_zeros() for _ in range(pipeline_n)]
        t0 = time.perf_counter()
        all_outs = [sharded(*dev_in, *zs) for zs in zss]
        jax.block_until_ready(all_outs)
        state["outs"] = all_outs[-1]
        return int((time.perf_counter() - t0) * 1e9 / pipeline_n)

    def get_y():
        y = np.asarray(state["outs"][out_names.index("Y")])  # [8, BC]
        return np.ascontiguousarray(
            y.reshape(1, B_FULL).reshape(B_FULL, 1)).astype(np.float32)

    if iters is None:
        return run_once, get_y

    times = [run_once() for _ in range(iters)]
    return min(times), times, get_y()


# revision 11
# speedup vs baseline: 1.6783x; 1.0520x over previous
"""DGMNet (dense MLP, 4 DGM layers) Trainium2 kernel.

Strategy: data-parallel over the batch dim (65536 rows -> 8 cores x 8192).
Inside each core, activations live feature-major in SBUF; every gate
matmul is out[M=feat,N=batch] = W.T-slice @ S with PE accumulation over
the 1024-feature contraction.

Speed levers over the fp32 formulation:
  1. Layer-0 algebraic fold: S1 = x@Sw.T+b is affine in the 16-wide x, so
     the G (wgS1), layer-0 Z and layer-0 R pre-activations are affine in
     x too. Host-side we fold Wg@Sw and Wz@Sw into 1024x16 matrices, so
     those three 1024-contraction matmuls become K=16 matmuls. Only 10 of
     the original 12 big matmuls per batch-tile remain.
  2. fp8e4m3 DoubleRow matmuls for H0 and all of layers 1-2 (7 of the 10
     big gates): 2 k-tiles per MM at ~108 ns per 512-col-MM-equivalent,
     2x the fp16 rate (measured). Layer 3 stays fp16: quantization errors
     add in quadrature and the final layer contributes the most, so this
     config sims at rel-err 1.6e-2 vs the 2e-2 budget.
  3. Biases ride the matmul via a 17th ones-row of x (so U blocks are
     [17, 1024] with the bias as row 16), letting each PSUM pair evacuate
     with ONE bias-free ACT tanh over [128, 2, 512] (halves ACT
     instruction count; ACT has a 352-cycle fixed cost per op).

Host-side preprocessing (numpy): transpose x/weights, build the folded U
block (7 gates x [17,1024] incl. bias row) replicated at partitions
0/32/64/96 for row-tiled K=17 matmuls, fp8-quantize Wg/Wz into
[128, 8, 1024] k-tile-major layout.
"""

import sys

sys.path.insert(0, "/opt/trn_rl_repo")

import numpy as np

B_FULL = 65536
KI = 16
KI1 = 17               # x rows + ones row for bias
H = 1024
NCORES = 8
BC = B_FULL // NCORES  # per-core batch (8192)
NB = 512               # batch tile (one PSUM bank of fp32)
NM = H // 128          # feature tiles (8)
NP = NM // 2           # feature-tile pairs (4)
N_LAYERS = 4

MM_DT = "float16"
FP8 = True             # fp8 DoubleRow for H0 + layers 1-2
FP8_Z3 = True          # additionally run layer-3 Z in fp8 (err 1.77e-2)

# gate indices into the U block / bias table
G_S1, G_Z0, G_G, G_R0, G_Z, G_R, G_H = range(7)
NGATES = 7

_BUILD_CACHE = {}


def _build(bc, nb, mm_dt, repeat=1, fp8=FP8):
    """Build + compile the single-core Bass program. Returns nc.

    repeat > 1 re-runs the whole computation (for slope-based timing of the
    device execution under the large axon dispatch overhead)."""
    import concourse.bacc as bacc
    import concourse.mybir as mybir
    import concourse.tile as tile

    f32 = mybir.dt.float32
    mdt = getattr(mybir.dt, mm_dt)
    f8 = mybir.dt.float8e4
    DR = mybir.MatmulPerfMode.DoubleRow
    Tanh = mybir.ActivationFunctionType.Tanh
    mult = mybir.AluOpType.mult
    add = mybir.AluOpType.add

    nt = bc // nb

    nc = bacc.Bacc("TRN2", target_bir_lowering=False, debug=False,
                   num_devices=NCORES)

    xT_d = nc.dram_tensor("xT", [KI1, bc], mdt, kind="ExternalInput").ap()
    wz_d = nc.dram_tensor("WzT", [H, H], mdt, kind="ExternalInput").ap()
    wg_d = nc.dram_tensor("WgT", [H, H], mdt, kind="ExternalInput").ap()
    u_d = nc.dram_tensor("U", [128, NGATES * H], mdt,
                         kind="ExternalInput").ap()
    bias_d = nc.dram_tensor("BIAS", [1, 2], f32, kind="ExternalInput").ap()
    ow_d = nc.dram_tensor("OW", [128, NM], mdt, kind="ExternalInput").ap()
    if fp8:
        wg8_d = nc.dram_tensor("Wg8", [128, NM * H], f8,
                               kind="ExternalInput").ap()
        wz8_d = nc.dram_tensor("Wz8", [128, NM * H], f8,
                               kind="ExternalInput").ap()
    y_d = nc.dram_tensor("Y", [1, bc], f32, kind="ExternalOutput").ap()

    with tile.TileContext(nc) as tc:
        with (
            tc.tile_pool(name="const", bufs=1) as cpool,
            tc.tile_pool(name="xt", bufs=3) as xt_pool,
            tc.tile_pool(name="s", bufs=2) as s_pool,
            tc.tile_pool(name="act", bufs=1) as act_pool,
            tc.tile_pool(name="ov", bufs=2) as ov_pool,
            tc.tile_pool(name="psum", bufs=3, space="PSUM") as ps_pool,
            tc.tile_pool(name="pso", bufs=1, space="PSUM") as pso_pool,
        ):
            # ---- resident constants ------------------------------------
            u_sb = cpool.tile([128, NGATES * H], mdt)
            nc.gpsimd.dma_start(u_sb[:], u_d[:])
            bias_sb = cpool.tile([1, 2], f32)
            nc.gpsimd.dma_start(bias_sb[:], bias_d[:])
            ow_sb = cpool.tile([128, NM], mdt)
            nc.gpsimd.dma_start(ow_sb[:], ow_d[:])
            if fp8:
                wg8_sb = cpool.tile([128, NM, H], f8)
                nc.gpsimd.dma_start(wg8_sb[:, :, :], wg8_d[:])
                wz8_sb = cpool.tile([128, NM, H], f8)
                nc.gpsimd.dma_start(wz8_sb[:, :, :], wz8_d[:])
            wg_sb = cpool.tile([128, NM * H], mdt)
            wz_sb = cpool.tile([128, NM * H], mdt)
            for k in range(NM):
                nc.gpsimd.dma_start(wg_sb[:, k * H:(k + 1) * H],
                                    wg_d[k * 128:(k + 1) * 128, :])
            for k in range(NM):
                nc.gpsimd.dma_start(wz_sb[:, k * H:(k + 1) * H],
                                    wz_d[k * 128:(k + 1) * 128, :])

            def w_ap(w_sb, k, m):
                return w_sb[:, k * H + m * 128:k * H + (m + 1) * 128]

            def u_ap(g, m, c):
                return u_sb[32 * c:32 * c + KI1,
                            g * H + m * 128:g * H + (m + 1) * 128]

            def k17_quad(gate, xt, jq, single, nametag):
                """Two [128,2,nb] pair-PSUMs for j=jq,jq+1 with row-tiled
                K=17 start matmuls (bias rides row 16 of xt/U)."""
                pps = []
                for j in (jq, jq + 1):
                    pp = ps_pool.tile([128, 2, nb], f32, tag="ps",
                                      name=f"{nametag}_{j}")
                    pps.append(pp)
                for idx, j in enumerate((jq, jq + 1)):
                    for h2 in range(2):
                        m = 2 * j + h2
                        c = m % 4
                        nc.tensor.matmul(
                            pps[idx][:, h2:h2 + 1, :], u_ap(gate, m, c),
                            xt[32 * c:32 * c + KI1, :],
                            start=True, stop=single,
                            tile_position=(32 * c, 0))
                return pps

            def small_gate(gate, xt, dests, act, nametag):
                """K=17-only gate (folded): quad starts + pair evacuation."""
                for jq in (0, 2):
                    pps = k17_quad(gate, xt, jq, True, nametag)
                    for idx, j in enumerate((jq, jq + 1)):
                        if act is None:
                            nc.vector.tensor_copy(dests[j][:, :, :],
                                                  pps[idx][:, :, :])
                        else:
                            nc.scalar.activation(dests[j][:, :, :],
                                                 pps[idx][:, :, :], act)

            def big_gate8(gate, xt, w8, rhs8, dests, nametag):
                """fp8 DoubleRow gate: K=17 fp16 start + 4 DR matmuls (2
                k-tiles each) per m, pair-fused tanh evacuation."""
                for jq in (0, 2):
                    pps = k17_quad(gate, xt, jq, False, nametag)
                    for idx, j in enumerate((jq, jq + 1)):
                        for h2 in range(2):
                            m = 2 * j + h2
                            for kj in range(4):
                                nc.tensor.matmul(
                                    pps[idx][:, h2:h2 + 1, :],
                                    w8[:, 2 * kj:2 * kj + 2,
                                       m * 128:(m + 1) * 128],
                                    rhs8[:, 2 * kj:2 * kj + 2, :],
                                    start=False, stop=(kj == 3),
                                    perf_mode=DR)
                        nc.scalar.activation(dests[j][:, :, :],
                                             pps[idx][:, :, :], Tanh)

            def big_gate16(gate, xt, w_sb, rhs_pairs, dests, nametag):
                """fp16 gate: K=17 start + 8 k-tile matmuls per m."""
                for jq in (0, 2):
                    pps = k17_quad(gate, xt, jq, False, nametag)
                    for idx, j in enumerate((jq, jq + 1)):
                        for h2 in range(2):
                            m = 2 * j + h2
                            for k in range(NM):
                                nc.tensor.matmul(
                                    pps[idx][:, h2:h2 + 1, :],
                                    w_ap(w_sb, k, m),
                                    rhs_pairs[k // 2][:, k % 2:k % 2 + 1, :],
                                    start=False, stop=(k == NM - 1))
                        nc.scalar.activation(dests[j][:, :, :],
                                             pps[idx][:, :, :], Tanh)

            # ---- per batch tile -----------------------------------------
            pend = None  # deferred output row of the previous batch tile

            def emit_out(pend):
                h_prev, tp, up = pend
                po = pso_pool.tile([1, nb], f32, tag="po", name=f"po_{up}")
                for k in range(NM):
                    nc.tensor.matmul(po[:], ow_sb[:, k:k + 1],
                                     h_prev[k // 2][:, k % 2:k % 2 + 1, :],
                                     start=(k == 0), stop=(k == NM - 1))
                orow = ov_pool.tile([1, nb], f32, tag="orow", name=f"orow_{up}")
                nc.vector.tensor_scalar_add(orow[:], po[:],
                                            bias_sb[0:1, 0:1])
                nc.gpsimd.dma_start(y_d[0:1, tp * nb:(tp + 1) * nb], orow[:])

            def pair_tiles(tag, t_u, i, dt_):
                return [act_pool.tile([128, 2, nb], dt_, tag=f"{tag}{j}",
                                      name=f"{tag}_{t_u}_{i}_{j}")
                        for j in range(NP)]

            for rep in range(repeat):
                for t in range(nt):
                    t_u = rep * nt + t  # unique suffix for tile names
                    xt = xt_pool.tile([128, nb], mdt, tag="xt",
                                      name=f"xt_{t_u}")
                    for c in range(4):
                        nc.gpsimd.dma_start(xt[32 * c:32 * c + KI1, :],
                                            xT_d[:, t * nb:(t + 1) * nb])

                    # S1 = x @ Sw.T + b (raw; DVE copy evacuation)
                    s_cur = [s_pool.tile([128, 2, nb], mdt, tag=f"s{j}",
                                         name=f"s_{t_u}_0_{j}")
                             for j in range(NP)]
                    small_gate(G_S1, xt, s_cur, None, f"ps_s1_{t_u}")

                    # G = tanh((Ug + Wg Sw) x + b') -- folded, K=17 only.
                    # Loop-invariant across layers; (1-G) deferred until
                    # H0's matmuls are in flight.
                    g_t = pair_tiles("g", t_u, 0, mdt)
                    small_gate(G_G, xt, g_t, Tanh, f"ps_g_{t_u}")

                    # Z0 / R0: folded, K=17 only
                    z_t = pair_tiles("z", t_u, 0, mdt)
                    small_gate(G_Z0, xt, z_t, Tanh, f"ps_z0_{t_u}")
                    r_t = pair_tiles("r", t_u, 0, mdt)
                    small_gate(G_R0, xt, r_t, Tanh, f"ps_r0_{t_u}")
                    # Z*S computed early (off the post-H critical chain)
                    for j in range(NP):
                        nc.vector.tensor_mul(z_t[j][:, :, :], z_t[j][:, :, :],
                                             s_cur[j][:, :, :])

                    # previous tile's output row fills the dep gap
                    if pend is not None:
                        emit_out(pend)
                        pend = None

                    for i in range(N_LAYERS):
                        use8 = fp8 and i < N_LAYERS - 1
                        if i > 0:
                            r_t = pair_tiles("r", t_u, i, mdt)
                            z_t = pair_tiles("z", t_u, i, mdt)
                            if use8:
                                big_gate8(G_R, xt, wg8_sb, s8, r_t,
                                          f"ps_r_{t_u}_{i}")
                                big_gate8(G_Z, xt, wz8_sb, s8, z_t,
                                          f"ps_z_{t_u}_{i}")
                            else:
                                big_gate16(G_R, xt, wg_sb, s_cur, r_t,
                                           f"ps_r_{t_u}_{i}")
                                if fp8 and FP8_Z3:
                                    big_gate8(G_Z, xt, wz8_sb, s8, z_t,
                                              f"ps_z_{t_u}_{i}")
                                else:
                                    big_gate16(G_Z, xt, wz_sb, s_cur, z_t,
                                               f"ps_z_{t_u}_{i}")
                            # Z*S early (off the post-H critical chain)
                            for j in range(NP):
                                nc.vector.tensor_mul(z_t[j][:, :, :],
                                                     z_t[j][:, :, :],
                                                     s_cur[j][:, :, :])

                        # SR = S * R
                        h_t = pair_tiles("h", t_u, i, mdt)
                        if use8:
                            sr8 = act_pool.tile([128, NM, nb], f8,
                                                tag="sr8", bufs=2,
                                                name=f"sr8_{t_u}_{i}")
                            for j in range(NP):
                                nc.vector.tensor_mul(
                                    sr8[:, 2 * j:2 * j + 2, :],
                                    s_cur[j][:, :, :], r_t[j][:, :, :])
                            big_gate8(G_H, xt, wg8_sb, sr8, h_t,
                                      f"ps_h_{t_u}_{i}")
                        else:
                            for j in range(NP):
                                nc.vector.tensor_mul(r_t[j][:, :, :],
                                                     s_cur[j][:, :, :],
                                                     r_t[j][:, :, :])
                            big_gate16(G_H, xt, wg_sb, r_t, h_t,
                                       f"ps_h_{t_u}_{i}")

                        if i == 0:
                            # deferred (1 - G), now that H0's matmuls are
                            # in flight
                            for j in range(NP):
                                nc.vector.tensor_scalar(g_t[j][:, :, :],
                                                        g_t[j][:, :, :],
                                                        -1.0, 1.0,
                                                        op0=mult, op1=add)

                        # output = (1-G)*H + Z*S  (Z*S already in z_t)
                        for j in range(NP):
                            nc.vector.tensor_mul(h_t[j][:, :, :],
                                                 g_t[j][:, :, :],
                                                 h_t[j][:, :, :])
                            nc.vector.tensor_add(h_t[j][:, :, :],
                                                 h_t[j][:, :, :],
                                                 z_t[j][:, :, :])

                        if i < N_LAYERS - 1:
                            s_new = [s_pool.tile([128, 2, nb], mdt,
                                                 tag=f"s{j}",
                                                 name=f"s_{t_u}_{i + 1}_{j}")
                                     for j in range(NP)]
                            for j in range(NP):
                                nc.scalar.activation(s_new[j][:, :, :],
                                                     h_t[j][:, :, :], Tanh)
                            if fp8 and (i < N_LAYERS - 2 or FP8_Z3):
                                # fp8 copy of S for next layer's R/Z rhs
                                s8 = act_pool.tile([128, NM, nb], f8,
                                                   tag="s8", bufs=2,
                                                   name=f"s8_{t_u}_{i + 1}")
                                for j in range(NP):
                                    nc.scalar.activation(
                                        s8[:, 2 * j:2 * j + 2, :],
                                        h_t[j][:, :, :], Tanh)
                            s_cur = s_new

                    # y = out_w @ output + out_b, deferred into the next
                    # tile's start phase
                    pend = (h_t, t, t_u)

            if pend is not None:
                emit_out(pend)

    nc.compile()
    return nc


def _get_nc(bc=BC, nb=NB, mm_dt=MM_DT):
    key = (bc, nb, mm_dt)
    if key not in _BUILD_CACHE:
        _BUILD_CACHE[key] = _build(bc, nb, mm_dt)
    return _BUILD_CACHE[key]


def _prep_inputs(x, Sw_w, Sw_b, Uz_w, Uz_b, Wz_w, Wz_b, Ug_w, Ug_b, Wg_w,
                 Wg_b, Ur_w, Ur_b, Uh_w, Uh_b, out_w, out_b):
    import ml_dtypes
    from concourse import mybir

    f = np.float32
    h = np.float16
    f8 = mybir.dt.np(mybir.dt.float8e4)
    Sw = np.asarray(Sw_w, f)
    Wz = np.asarray(Wz_w, f)
    Wg = np.asarray(Wg_w, f)
    WzSw = Wz @ Sw                                          # [H, 16]
    WgSw = Wg @ Sw
    xT = np.ones((KI1, B_FULL), h)
    xT[:KI] = np.asarray(x, f).T.astype(h)                  # row 16 stays 1.0
    WzT = np.ascontiguousarray(Wz.T).astype(h)              # [H, H]
    WgT = np.ascontiguousarray(Wg.T).astype(h)
    # fp8 copies in [128, k, H] k-tile-major layout
    Wg8 = np.ascontiguousarray(
        WgT.reshape(NM, 128, H).transpose(1, 0, 2).reshape(128, NM * H)
    ).astype(f8)
    Wz8 = np.ascontiguousarray(
        WzT.reshape(NM, 128, H).transpose(1, 0, 2).reshape(128, NM * H)
    ).astype(f8)
    WzSb = Wz @ np.asarray(Sw_b, f)
    WgSb = Wg @ np.asarray(Sw_b, f)
    gates_U = [
        (Sw, np.asarray(Sw_b, f)),                           # S1
        (np.asarray(Uz_w, f) + WzSw,
         np.asarray(Uz_b, f) + np.asarray(Wz_b, f) + WzSb),  # Z0 folded
        (np.asarray(Ug_w, f) + WgSw,
         np.asarray(Ug_b, f) + np.asarray(Wg_b, f) + WgSb),  # G folded
        (np.asarray(Ur_w, f) + WgSw,
         np.asarray(Ur_b, f) + np.asarray(Wg_b, f) + WgSb),  # R0 folded
        (np.asarray(Uz_w, f),
         np.asarray(Uz_b, f) + np.asarray(Wz_b, f)),         # Z
        (np.asarray(Ur_w, f),
         np.asarray(Ur_b, f) + np.asarray(Wg_b, f)),         # R
        (np.asarray(Uh_w, f),
         np.asarray(Uh_b, f) + np.asarray(Wg_b, f)),         # H
    ]
    U17 = np.concatenate(
        [np.concatenate([w.T, b.reshape(1, H)], axis=0) for w, b in gates_U],
        axis=1)                                              # [17, 7H]
    U = np.zeros((128, NGATES * H), h)
    for c in range(4):
        U[32 * c:32 * c + KI1] = U17.astype(h)
    bias = np.zeros((1, 2), f)
    bias[0, 0] = np.float32(np.asarray(out_b, f)[0])
    OW = np.ascontiguousarray(
        np.asarray(out_w, f).reshape(NM, 128).T).astype(h)
    return xT, WzT, WgT, U, bias, OW, Wg8, Wz8


def kernel(**inputs):
    from concourse.bass_utils import run_bass_kernel_spmd

    nc = _get_nc()
    in_maps = _make_in_maps(inputs)
    res = run_bass_kernel_spmd(nc, in_maps, list(range(NCORES)))
    y = np.concatenate([res.results[c]["Y"] for c in range(NCORES)], axis=1)
    return np.ascontiguousarray(y.reshape(B_FULL, 1)).astype(np.float32)


def _make_in_maps(inputs):
    xT, WzT, WgT, U, bias, OW, Wg8, Wz8 = _prep_inputs(**inputs)
    return [{
        "xT": np.ascontiguousarray(xT[:, c * BC:(c + 1) * BC]),
        "WzT": WzT, "WgT": WgT, "U": U, "BIAS": bias, "OW": OW,
        "Wg8": Wg8, "Wz8": Wz8,
    } for c in range(NCORES)]


def timed_run(inputs, iters=5, nc=None, pipeline=1):
    """Build a persistent jitted runner (so walrus compiles once), stage the
    inputs on-device, and time repeated executions. Returns (best_ns,
    all_ns, output)."""
    import time
    import jax
    from jax.sharding import Mesh, PartitionSpec, NamedSharding
    from jax.experimental.shard_map import shard_map
    from concourse import bass2jax, mybir

    bass2jax.install_neuronx_cc_hook()
    if nc is None:
        nc = _get_nc()
    in_maps = _make_in_maps(inputs)
    n_cores = NCORES

    partition_name = (nc.partition_id_tensor.name
                      if nc.partition_id_tensor else None)
    in_names, out_names, out_avals, zero_outs = [], [], [], []
    for alloc in nc.m.functions[0].allocations:
        if not isinstance(alloc, mybir.MemoryLocationSet):
            continue
        name = alloc.memorylocations[0].name
        if alloc.kind == "ExternalInput":
            if name != partition_name:
                in_names.append(name)
        elif alloc.kind == "ExternalOutput":
            shape = tuple(alloc.tensor_shape)
            dtype = mybir.dt.np(alloc.dtype)
            out_names.append(name)
            out_avals.append(jax.core.ShapedArray(shape, dtype))
            zero_outs.append(np.zeros(shape, dtype))
    n_params = len(in_names)
    n_outs = len(out_avals)
    all_in = list(in_names) + list(out_names)
    if partition_name is not None:
        all_in.append(partition_name)
    donate = tuple(range(n_params, n_params + n_outs))

    def _body(*args):
        operands = list(args)
        if partition_name is not None:
            operands.append(bass2jax.partition_id_tensor())
        outs = bass2jax._bass_exec_p.bind(
            *operands,
            out_avals=tuple(out_avals),
            in_names=tuple(all_in),
            out_names=tuple(out_names),
            lowering_input_output_aliases=(),
            sim_require_finite=True,
            sim_require_nnan=True,
            nc=nc,
        )
        return tuple(outs)

    devices = jax.devices()[:n_cores]
    mesh = Mesh(np.asarray(devices), ("core",))
    spec = PartitionSpec("core")
    sharded = jax.jit(
        shard_map(_body, mesh=mesh, in_specs=(spec,) * (n_params + n_outs),
                  out_specs=(spec,) * n_outs, check_rep=False),
        donate_argnums=donate, keep_unused=True)

    sharding = NamedSharding(mesh, spec)
    dev_in = [
        jax.device_put(
            np.concatenate([np.asarray(in_maps[c][n]) for c in range(n_cores)],
                           axis=0), sharding)
        for n in in_names
    ]
    def fresh_zeros():
        return [np.zeros((n_cores * z.shape[0], *z.shape[1:]), z.dtype)
                for z in zero_outs]

    # warmup (compiles)
    outs = sharded(*dev_in, *fresh_zeros())
    jax.block_until_ready(outs)

    state = {"outs": outs}

    def run_once(pipeline_n=pipeline):
        zss = [fresh_zeros() for _ in range(pipeline_n)]
        t0 = time.perf_counter()
        all_outs = [sharded(*dev_in, *zs) for zs in zss]
        jax.block_until_ready(all_outs)
        state["outs"] = all_outs[-1]
        return int((time.perf_counter() - t0) * 1e9 / pipeline_n)

    def get_y():
        y = np.asarray(state["outs"][out_names.index("Y")])  # [8, BC]
        return np.ascontiguousarray(
            y.reshape(1, B_FULL).reshape(B_FULL, 1)).astype(np.float32)

    if iters is None:
        return run_once, get_y

    times = [run_once() for _ in range(iters)]
    return min(times), times, get_y()
